# revision 1
# baseline (speedup 1.0000x reference)
"""Trainium2 Bass kernel for nn_ReachabilityClassifierTransformer.

Data-parallel over batch: 16 samples / 8 cores = 2 samples per core.
Each core runs the full network (6-layer encoder + 4-layer decoder + head)
on its 2 samples. No collectives.

Device layout conventions (per core):
  - Activations are kept FEATURE-MAJOR in SBUF: tile [128, KC, T] holds
    X.T, i.e. element [p, k, t] = X[t, k*128+p]. T = 2*512 tokens
    (sample-major concat).
  - All weights are pre-transposed on host to [in_feat, out_feat] and laid
    out as [128, KC_in, O] (partition = in-feature % 128).
  - matmul(out_psum[M,N], lhsT=[K,M], rhs=[K,N]) computes lhsT.T @ rhs with
    K on partitions.  "Option B": lhsT = weight chunk -> output feature-major.
    "Option A": lhsT = activation chunk -> output token-major (used for V).
  - float32r is used for every matmul operand (full-rate fp32 on the PE).
  - Encoder stage-1 q,k output features are de-interleaved (even feats then
    odd feats) via host-side column permutation of in_proj, so RoPE becomes
    contiguous block ops; the roped result is in natural order again.
  - Softmax: scores are computed transposed (S.T = K_h @ Q_h.T per 128-row
    chunk), exp'd without max subtraction (|scores/8| < 1 for this model),
    and the denominator comes free from a ones-column appended to V.
"""
import numpy as np

import concourse.bass as bass
import concourse.mybir as mybir
import concourse.tile as tile
from concourse import bacc
from concourse.bass_utils import run_bass_kernel_spmd

AF = mybir.ActivationFunctionType
ALU = mybir.AluOpType
F32 = mybir.dt.float32
F32R = mybir.dt.float32r
F16 = mybir.dt.float16

B, S, D, FF, H, LE, LD, M = 16, 512, 512, 2048, 8, 6, 4, 2048
ROPE_BASE = 10000.0
LN_EPS = 1e-5
NCORES = 8
BL = B // NCORES          # 2 samples per core
T = BL * S                # 1024 tokens per core
KC = D // 128             # 4 feature chunks
FC = FF // 128            # 16
MC = M // 128             # 16
DH = D // H               # 64


# ----------------------------------------------------------------------------
# host-side helpers
# ----------------------------------------------------------------------------

def _chunked(wT):
    """[Din, O] -> [128, Din//128, O] contiguous."""
    Din, O = wT.shape
    return np.ascontiguousarray(
        wT.reshape(Din // 128, 128, O).transpose(1, 0, 2)).astype(np.float32)


def _bias_cols(b):
    """[O] -> [128, O//128]  (column per 128-chunk)."""
    O = b.shape[0]
    return np.ascontiguousarray(b.reshape(O // 128, 128).T).astype(np.float32)


_DEINT = np.concatenate([np.arange(0, D, 2), np.arange(1, D, 2)])  # de-interleave


def prep_weights(inp, le=LE, ld=LD):
    """Host-side weight prep -> dict of arrays shared by all cores."""
    out = {}
    g = {k: np.asarray(v, np.float32) for k, v in inp.items()}

    out["mpwT"] = np.ascontiguousarray(g["morph_proj_w"].T)        # [3, 512]
    out["mpb"] = _bias_cols(g["morph_proj_b"])                     # [128, 4]
    out["ppwT"] = np.ascontiguousarray(g["pose_proj_w"].T)         # [9, 512]
    out["ppb"] = _bias_cols(g["pose_proj_b"])

    # rope grids, de-interleaved frequency order: [128, 2, 512]
    freq = 1.0 / ROPE_BASE ** (np.arange(0, D, 2, dtype=np.float64) / D)
    ang = np.outer(np.arange(S, dtype=np.float64), freq)           # [512, 256]
    out["gridc"] = _chunked(np.cos(ang).T.astype(np.float32).reshape(256, S)).astype(np.float16)
    out["grids"] = _chunked(np.sin(ang).T.astype(np.float32).reshape(256, S)).astype(np.float16)

    e_w1, e_w1b, e_w2, e_w2b, e_vb = [], [], [], [], []
    e_ow, e_owb, e_l1, e_l1b, e_l2, e_l2b = [], [], [], [], [], []
    for i in range(le):
        w1 = g["enc_in_w"][i] * g["enc_n1_g"][i][None, :]          # fold n1 g
        b1 = g["enc_in_b"][i] + g["enc_in_w"][i] @ g["enc_n1_b"][i]
        # stage-1: de-interleave q,k output columns
        perm = np.concatenate([_DEINT, D + _DEINT, 2 * D + np.arange(D)])
        e_w1.append(_chunked(np.ascontiguousarray(w1[perm].T)))    # [128,4,1536]
        e_w1b.append(_bias_cols(b1[perm]))                         # [128,12]
        # stage-2 (natural order, raw weights - the faithful quirk)
        w2 = g["enc_in_w"][i][: 2 * D]                             # Wq;Wk
        e_w2.append(_chunked(np.ascontiguousarray(w2.T)))          # [128,4,1024]
        e_w2b.append(_bias_cols(g["enc_in_b"][i][: 2 * D]))        # [128,8]
        e_vb.append(g["enc_in_b"][i][2 * D:][None, :])             # [1,512]
        e_ow.append(_chunked(np.ascontiguousarray(g["enc_out_w"][i].T)))
        e_owb.append(_bias_cols(g["enc_out_b"][i]))
        l1 = g["enc_l1_w"][i] * g["enc_n2_g"][i][None, :]
        l1b = g["enc_l1_b"][i] + g["enc_l1_w"][i] @ g["enc_n2_b"][i]
        e_l1.append(_chunked(np.ascontiguousarray(l1.T)))          # [128,4,2048]
        e_l1b.append(_bias_cols(l1b))                              # [128,16]
        e_l2.append(_chunked(np.ascontiguousarray(g["enc_l2_w"][i].T)))
        e_l2b.append(_bias_cols(g["enc_l2_b"][i]))                 # [128,4]
    out["ew1T"], out["ew1b"] = np.stack(e_w1), np.stack(e_w1b)
    out["ew2T"], out["ew2b"] = np.stack(e_w2), np.stack(e_w2b)
    out["evb"] = np.stack(e_vb)
    out["eowT"], out["eowb"] = np.stack(e_ow), np.stack(e_owb)
    out["el1T"], out["el1b"] = np.stack(e_l1), np.stack(e_l1b)
    out["el2T"], out["el2b"] = np.stack(e_l2), np.stack(e_l2b)

    d_in, d_inb, d_vb, d_ow, d_owb = [], [], [], [], []
    d_m1, d_m1b, d_m2, d_m2b = [], [], [], []
    for i in range(ld):
        w = g["dec_in_w"][i].copy()
        b = g["dec_in_b"][i].copy()
        w[:D] = w[:D] * g["dec_n1_g"][i][None, :]                  # Wq <- dec_n1
        b[:D] = b[:D] + g["dec_in_w"][i][:D] @ g["dec_n1_b"][i]
        w[D:] = w[D:] * g["enc_final_g"][None, :]                  # Wk,Wv <- enc_final
        b[D:] = b[D:] + g["dec_in_w"][i][D:] @ g["enc_final_b"]
        d_in.append(_chunked(np.ascontiguousarray(w.T)))           # [128,4,1536]
        d_inb.append(_bias_cols(b))
        d_vb.append(b[2 * D:][None, :])                            # [1,512]
        d_ow.append(_chunked(np.ascontiguousarray(g["dec_out_w"][i].T)))
        d_owb.append(_bias_cols(g["dec_out_b"][i]))
        m1 = g["dec_m1_w"][i] * g["dec_n2_g"][i][None, :]
        m1b = g["dec_m1_b"][i] + g["dec_m1_w"][i] @ g["dec_n2_b"][i]
        d_m1.append(_chunked(np.ascontiguousarray(m1.T)))          # [128,4,2048]
        d_m1b.append(_bias_cols(m1b))
        d_m2.append(_chunked(np.ascontiguousarray(g["dec_m2_w"][i].T)))
        d_m2b.append(_bias_cols(g["dec_m2_b"][i]))
    out["dinT"], out["dinb"] = np.stack(d_in), np.stack(d_inb)
    out["dvb"] = np.stack(d_vb)
    out["dowT"], out["dowb"] = np.stack(d_ow), np.stack(d_owb)
    out["dm1T"], out["dm1b"] = np.stack(d_m1), np.stack(d_m1b)
    out["dm2T"], out["dm2b"] = np.stack(d_m2), np.stack(d_m2b)

    hw = (g["head_w"] * g["head_g"][None, :])[0]                   # [512]
    out["hwT"] = _bias_cols(hw)                                    # [128, 4]
    out["hb"] = (g["head_bias"] + g["head_w"] @ g["head_b"]).reshape(1, 1)
    return out


# ----------------------------------------------------------------------------
# device program
# ----------------------------------------------------------------------------

def build(le=LE, ld=LD):
    nc = bacc.Bacc(None, target_bir_lowering=False)

    dram = {}

    def din(name, shape, dt=F32R):
        dram[name] = nc.dram_tensor(name, list(shape), dt, kind="ExternalInput")
        return dram[name]

    # shared weights
    din("mpwT", [3, 512]); din("mpb", [128, 4], F32)
    din("ppwT", [9, 512]); din("ppb", [128, 4], F32)
    din("gridc", [128, 2, S], F16); din("grids", [128, 2, S], F16)
    din("ew1T", [le, 128, KC, 3 * D]); din("ew1b", [le, 128, 12], F32)
    din("ew2T", [le, 128, KC, 2 * D]); din("ew2b", [le, 128, 8], F32)
    din("evb", [le, 1, D])
    din("eowT", [le, 128, KC, D]); din("eowb", [le, 128, 4], F32)
    din("el1T", [le, 128, KC, FF]); din("el1b", [le, 128, 16], F32)
    din("el2T", [le, 128, FC, D]); din("el2b", [le, 128, 4], F32)
    din("dinT", [ld, 128, KC, 3 * D]); din("dinb", [ld, 128, 12], F32)
    din("dvb", [ld, 1, D])
    din("dowT", [ld, 128, KC, D]); din("dowb", [ld, 128, 4], F32)
    din("dm1T", [ld, 128, KC, M]); din("dm1b", [ld, 128, 16], F32)
    din("dm2T", [ld, 128, MC, D]); din("dm2b", [ld, 128, 4], F32)
    din("hwT", [128, KC]); din("hb", [1, 1], F32)
    # per-core inputs
    din("morphT", [3, T])
    din("poseT", [9, BL])
    y = nc.dram_tensor("y", [1, BL], F32, kind="ExternalOutput")

    with tile.TileContext(nc) as tc:
        _build_body(nc, tc, dram, y, le, ld)
    nc.compile()
    return nc


def _build_body(nc, tc, dram, y_dram, le, ld):
    import contextlib
    ctx = contextlib.ExitStack()
    with ctx:
        ctx.enter_context(nc.allow_low_precision(
            reason="float32r rounding of matmul operands is intentional"))
        persist = ctx.enter_context(tc.tile_pool(name="persist", bufs=1))
        wpool = ctx.enter_context(tc.tile_pool(name="wpool", bufs=2))
        w2pool = ctx.enter_context(tc.tile_pool(name="w2pool", bufs=1))
        owpool = ctx.enter_context(tc.tile_pool(name="owpool", bufs=1))
        bpool = ctx.enter_context(tc.tile_pool(name="bpool", bufs=2))
        a4 = ctx.enter_context(tc.tile_pool(name="a4", bufs=3))
        a8 = ctx.enter_context(tc.tile_pool(name="a8", bufs=2))
        vp = ctx.enter_context(tc.tile_pool(name="vp", bufs=1))
        rtp = ctx.enter_context(tc.tile_pool(name="rtp", bufs=3))
        vbp = ctx.enter_context(tc.tile_pool(name="vbp", bufs=1))
        scr = ctx.enter_context(tc.tile_pool(name="scr", bufs=2))
        scrrc = ctx.enter_context(tc.tile_pool(name="scrrc", bufs=1))
        smalls = ctx.enter_context(tc.tile_pool(name="smalls", bufs=2))
        b1 = ctx.enter_context(tc.tile_pool(name="b1", bufs=4, space="PSUM"))
        b2 = ctx.enter_context(tc.tile_pool(name="b2", bufs=2, space="PSUM"))
        qk2p, atp = a8, a4  # share slots/tags

        # ---------------- persistent tiles ----------------
        x = persist.tile([128, KC, T], F32R)          # residual stream (X.T)
        gridc = persist.tile([128, 2, S], F16)
        grids = persist.tile([128, 2, S], F16)
        ones128 = persist.tile([128, 1], F32R)
        ones_row = persist.tile([1, 128], F32R)
        eps_t = persist.tile([1, 1], F32)
        p = persist.tile([128, KC, BL], F32R)         # decoder latent p.T
        nc.sync.dma_start(gridc[:], dram["gridc"][:])
        nc.sync.dma_start(grids[:], dram["grids"][:])
        ones8 = persist.tile([128, 8], F32R)
        stage_f32 = rtp.tile([128, 128], F32, tag="rt")
        nc.vector.memset(stage_f32[:], 1.0)
        nc.vector.tensor_copy(ones128[:], stage_f32[:, 0:1])
        nc.vector.tensor_copy(ones_row[:], stage_f32[0:1, :])
        nc.vector.tensor_copy(ones8[:], stage_f32[:, 0:8])
        nc.vector.memset(eps_t[:], LN_EPS)

        def c32(ap):
            return ap.bitcast(F32)

        def ln(x_tile, sl, n_tok, h_out, out_sl):
            """h_out[:, :, out_sl] = LayerNorm_features(x_tile[:, :, sl])."""
            sq = a4.tile([128, KC, n_tok], F32R, tag="a4")
            for k in range(KC):
                nc.scalar.activation(sq[:, k, :], x_tile[:, k, sl], AF.Square)
            sum_ps = b2.tile([1, n_tok], F32, tag="b2")
            sq_ps = b2.tile([1, n_tok], F32, tag="b2")
            cv = (lambda ap: ap) if n_tok >= 256 else c32
            for k in range(KC):
                nc.tensor.matmul(sum_ps[:], cv(ones128[:]), cv(x_tile[:, k, sl]),
                                 start=(k == 0), stop=(k == KC - 1))
            for k in range(KC):
                nc.tensor.matmul(sq_ps[:], cv(ones128[:]), cv(sq[:, k, :]),
                                 start=(k == 0), stop=(k == KC - 1))
            ms = scr.tile([1, n_tok], F32, tag="scr")
            t2 = scr.tile([1, n_tok], F32, tag="scr")
            rc = scrrc.tile([1, 2, n_tok], F32R, tag="scr_rc")
            nc.scalar.activation(ms[:], sum_ps[:], AF.Copy, scale=1.0 / D)
            # t2 = E[x^2] - m^2  (stt: (ms * ms) subtracted via reverse op)
            nc.vector.tensor_tensor(t2[:], ms[:], ms[:], ALU.mult)      # m^2
            nc.vector.scalar_tensor_tensor(
                t2[:], sq_ps[:], 1.0 / D, t2[:], ALU.mult, ALU.subtract)
            # t2 = sqrt(var + eps)
            nc.scalar.activation(t2[:], t2[:], AF.Sqrt, bias=eps_t[:])
            nc.vector.reciprocal(rc[:, 0, :], t2[:])                    # r
            nc.vector.scalar_tensor_tensor(
                rc[:, 1, :], ms[:], -1.0, rc[:, 0, :], ALU.mult, ALU.mult)  # c
            r_ps = b1.tile([128, n_tok], F32, tag="b1")
            c_ps = b1.tile([128, n_tok], F32, tag="b1")
            nc.tensor.matmul(r_ps[:], cv(ones_row[:]), cv(rc[:, 0, :]),
                             start=True, stop=True)
            nc.tensor.matmul(c_ps[:], cv(ones_row[:]), cv(rc[:, 1, :]),
                             start=True, stop=True)
            for k in range(KC):
                nc.vector.tensor_tensor(h_out[:, k, out_sl], x_tile[:, k, sl],
                                        r_ps[:], ALU.mult)
                nc.vector.tensor_tensor(h_out[:, k, out_sl], h_out[:, k, out_sl],
                                        c_ps[:], ALU.add)

        def ln_stats_pair(x_tile):
            """One LN chain for both samples: returns rc [1, 2, T] f32r."""
            sq = a8.tile([128, KC, T], F32R, tag="a8")
            for k in range(KC):
                nc.scalar.activation(sq[:, k, :], x_tile[:, k, :], AF.Square)
            sum_ps = b2.tile([1, T], F32, tag="b2")
            sq_ps = b2.tile([1, T], F32, tag="b2")
            for nh in range(BL):
                nsl = slice(nh * S, (nh + 1) * S)
                for k in range(KC):
                    nc.tensor.matmul(sum_ps[:, nsl], ones128[:],
                                     x_tile[:, k, nsl],
                                     start=(k == 0), stop=(k == KC - 1))
                for k in range(KC):
                    nc.tensor.matmul(sq_ps[:, nsl], ones128[:], sq[:, k, nsl],
                                     start=(k == 0), stop=(k == KC - 1))
            rc = scrrc.tile([1, 2, T], F32R, tag="scr_rcT")
            ms = rc[:, 1, :]                      # mean parked in the c slot
            t2 = scrrc.tile([1, T], F32, tag="scrT2")
            nc.scalar.activation(ms, sum_ps[:], AF.Copy, scale=1.0 / D)
            nc.vector.tensor_tensor(t2[:], ms, ms, ALU.mult)
            nc.vector.scalar_tensor_tensor(
                t2[:], sq_ps[:], 1.0 / D, t2[:], ALU.mult, ALU.subtract)
            nc.scalar.activation(t2[:], t2[:], AF.Sqrt, bias=eps_t[:])
            nc.vector.reciprocal(rc[:, 0, :], t2[:])
            nc.vector.scalar_tensor_tensor(
                rc[:, 1, :], ms, -1.0, rc[:, 0, :], ALU.mult, ALU.mult)
            return rc

        def ln_apply(rc, x_tile, s, h_out):
            sl = slice(s * S, (s + 1) * S)
            r_ps = b1.tile([128, S], F32, tag="b1")
            c_ps = b1.tile([128, S], F32, tag="b1")
            nc.tensor.matmul(r_ps[:], ones_row[:], rc[:, 0, sl],
                             start=True, stop=True)
            nc.tensor.matmul(c_ps[:], ones_row[:], rc[:, 1, sl],
                             start=True, stop=True)
            for k in range(KC):
                nc.vector.tensor_tensor(h_out[:, k, :], x_tile[:, k, sl],
                                        r_ps[:], ALU.mult)
                nc.vector.tensor_tensor(h_out[:, k, :], h_out[:, k, :],
                                        c_ps[:], ALU.add)

        # ---------------- morph projection -> x ----------------
        morpht = a8.tile([3, T], F32R, tag="a8")
        nc.sync.dma_start(morpht[:], dram["morphT"][:])
        mpw = rtp.tile([3, 512], F32R, tag="rt")
        mpb = rtp.tile([128, 4], F32, tag="rt")
        nc.sync.dma_start(mpw[:], dram["mpwT"][:])
        nc.sync.dma_start(mpb[:], dram["mpb"][:])
        for m in range(KC):
            for s in range(BL):
                ps = b1.tile([128, S], F32, tag="b1")
                nc.tensor.matmul(ps[:], mpw[:, m * 128:(m + 1) * 128],
                                 morpht[:, s * S:(s + 1) * S], start=True, stop=True)
                nc.scalar.activation(x[:, m, s * S:(s + 1) * S], ps[:], AF.Relu,
                                     bias=mpb[:, m:m + 1])

        # ---------------- pose projection -> p ----------------
        poset = rtp.tile([9, BL], F32R, tag="rt")
        ppw = rtp.tile([9, 512], F32R, tag="rt")
        ppb = rtp.tile([128, 4], F32, tag="rt")
        nc.sync.dma_start(poset[:], dram["poseT"][:])
        nc.sync.dma_start(ppw[:], dram["ppwT"][:])
        nc.sync.dma_start(ppb[:], dram["ppb"][:])
        pps = b1.tile([128, KC, BL], F32, tag="b1")
        for m in range(KC):
            nc.tensor.matmul(pps[:, m, :], c32(ppw[:, m * 128:(m + 1) * 128]),
                             c32(poset[:]), start=True, stop=True)
        for m in range(KC):
            nc.scalar.activation(p[:, m, :], pps[:, m, :], AF.Relu,
                                 bias=ppb[:, m:m + 1])

        # ---------------- encoder layers ----------------
        for li in range(le):
            w1 = wpool.tile([128, KC, 3 * D], F32R, tag="bigw")
            nc.sync.dma_start(w1[:], dram["ew1T"][li])
            w1b = bpool.tile([128, 12], F32, tag="w1b")
            nc.sync.dma_start(w1b[:], dram["ew1b"][li])
            w2 = w2pool.tile([128, KC, 2 * D], F32R, tag="w2")
            nc.sync.dma_start(w2[:], dram["ew2T"][li])
            w2b = bpool.tile([128, 8], F32, tag="w2b")
            nc.sync.dma_start(w2b[:], dram["ew2b"][li])
            vbrow = bpool.tile([1, D], F32R, tag="vbrow")
            nc.sync.dma_start(vbrow[:], dram["evb"][li])
            ow = owpool.tile([128, KC, D], F32R, tag="ow")
            nc.sync.dma_start(ow[:], dram["eowT"][li])
            owb = bpool.tile([128, 4], F32, tag="owb")
            nc.sync.dma_start(owb[:], dram["eowb"][li])
            l1 = wpool.tile([128, KC, FF], F32R, tag="bigw")
            nc.sync.dma_start(l1[:], dram["el1T"][li])
            l1b = bpool.tile([128, 16], F32, tag="l1b")
            nc.sync.dma_start(l1b[:], dram["el1b"][li])
            l2b = bpool.tile([128, 4], F32, tag="l2b")
            nc.sync.dma_start(l2b[:], dram["el2b"][li])

            # v-bias broadcast [128, 512] (token-major V bias), once per layer
            vb_ps = b1.tile([128, D], F32, tag="b1")
            nc.tensor.matmul(vb_ps[:], ones_row[:], vbrow[:], start=True, stop=True)
            vb_bc = vbp.tile([128, D], F32, tag="vb_bc")
            nc.scalar.activation(vb_bc[:], vb_ps[:], AF.Copy)

            rc1 = ln_stats_pair(x)
            for s in range(BL):
                sl = slice(s * S, (s + 1) * S)
                h = a4.tile([128, KC, S], F32R, tag="a4")
                ln_apply(rc1, x, s, h)
                # ---- stage 1: q,k (permuted) + v ----
                qkv1 = a8.tile([128, 8, S], F32R, tag="a8")
                v1 = a4.tile([128, KC, S], F32R, tag="a4")
                for m in range(12):
                    ps = b1.tile([128, S], F32, tag="b1")
                    for k in range(KC):
                        nc.tensor.matmul(ps[:], w1[:, k, m * 128:(m + 1) * 128],
                                         h[:, k, :], start=(k == 0), stop=(k == KC - 1))
                    dest = qkv1[:, m, :] if m < 8 else v1[:, m - 8, :]
                    nc.scalar.activation(dest, ps[:], AF.Identity,
                                         bias=w1b[:, m:m + 1])
                # ---- rope: qkv1 (chunks 0-7) -> qkr (natural order) ----
                qkr = a8.tile([128, 8, S], F32R, tag="a8")
                for half in (0, 4):
                    for c in range(2):
                        e = qkv1[:, half + c, :]
                        o = qkv1[:, half + 2 + c, :]
                        r1 = qkr[:, half + c, :]
                        r2 = qkr[:, half + 2 + c, :]
                        t1 = rtp.tile([128, S], F32, tag="rt")
                        nc.vector.tensor_tensor(r1, e, gridc[:, c, :], ALU.mult)
                        nc.vector.tensor_tensor(t1[:], o, grids[:, c, :], ALU.mult)
                        nc.vector.tensor_tensor(r1, r1, t1[:], ALU.subtract)
                        t2 = rtp.tile([128, S], F32, tag="rt")
                        nc.vector.tensor_tensor(r2, e, grids[:, c, :], ALU.mult)
                        nc.vector.tensor_tensor(t2[:], o, gridc[:, c, :], ALU.mult)
                        nc.vector.tensor_tensor(r2, r2, t2[:], ALU.add)
                # ---- stage 2: Q,K ----
                qk2 = qk2p.tile([128, 8, S], F32R, tag="a8")
                for m in range(8):
                    ps = b1.tile([128, S], F32, tag="b1")
                    base = 0 if m < 4 else 4
                    for k in range(KC):
                        nc.tensor.matmul(ps[:], w2[:, k, m * 128:(m + 1) * 128],
                                         qkr[:, base + k, :],
                                         start=(k == 0), stop=(k == KC - 1))
                    nc.scalar.activation(qk2[:, m, :], ps[:], AF.Identity,
                                         bias=w2b[:, m:m + 1])
                # ---- stage 2: V (token-major, with ones column per head) ----
                vloc = vp.tile([128, KC, 8, 65], F32R, tag="vloc")
                for t in range(KC):
                    nc.vector.tensor_copy(vloc[:, t, :, 64], ones8[:])
                for t in range(KC):
                    ps = b1.tile([128, S], F32, tag="b1")
                    for k in range(KC):
                        nc.tensor.matmul(
                            ps[:], v1[:, k, t * 128:(t + 1) * 128],
                            w1[:, k, 2 * D:3 * D],
                            start=(k == 0), stop=(k == KC - 1))
                    nc.vector.tensor_tensor(
                        vloc[:, t, :, 0:64],
                        ps[:].rearrange("p (h d) -> p h d", h=H),
                        vb_bc[:].rearrange("p (h d) -> p h d", h=H), ALU.add)
                # ---- attention heads (paired: exp(h+1) hides under A@V(h)) ----
                o_t = a4.tile([128, KC, S], F32R, tag="a4")
                for h0 in range(0, H, 2):
                    ats = {}
                    for hh in (h0, h0 + 1):
                        rows = slice(64 * (hh % 2), 64 * (hh % 2) + 64)
                        at = atp.tile([128, KC, S], F32R, tag="a4")
                        for c in range(KC):
                            scp = b1.tile([128, S], F32, tag="b1")
                            nc.tensor.matmul(
                                scp[:],
                                qk2[rows, 4 + hh // 2, c * 128:(c + 1) * 128],
                                qk2[rows, hh // 2, :], start=True, stop=True)
                            nc.scalar.activation(at[:, c, :], scp[:], AF.Exp,
                                                 scale=float(1.0 / np.sqrt(DH)))
                        ats[hh] = at
                    for hh in (h0, h0 + 1):
                        rows = slice(64 * (hh % 2), 64 * (hh % 2) + 64)
                        at = ats[hh]
                        ov = b2.tile([65, S], F32, tag="b2")
                        for c in range(KC):
                            nc.tensor.matmul(ov[:], vloc[:, c, hh, :], at[:, c, :],
                                             start=(c == 0), stop=(c == KC - 1))
                        rec = scr.tile([1, S], F32R, tag="scr")
                        nc.vector.reciprocal(rec[:], ov[64:65, :])
                        rb = b2.tile([64, S], F32, tag="b2")
                        nc.tensor.matmul(rb[:], ones_row[:, 0:64], rec[:],
                                         start=True, stop=True)
                        rb_sb = scr.tile([64, S], F32, tag="scr")
                        nc.scalar.activation(rb_sb[:], rb[:], AF.Copy)
                        nc.vector.tensor_tensor(o_t[rows, hh // 2, :],
                                                ov[0:64, :], rb_sb[:], ALU.mult)
                # ---- out-proj + residual ----
                for m in range(KC):
                    ps = b1.tile([128, S], F32, tag="b1")
                    for k in range(KC):
                        nc.tensor.matmul(ps[:], ow[:, k, m * 128:(m + 1) * 128],
                                         o_t[:, k, :], start=(k == 0),
                                         stop=(k == KC - 1))
                    nc.vector.scalar_tensor_tensor(
                        x[:, m, sl], ps[:], owb[:, m:m + 1], x[:, m, sl],
                        ALU.add, ALU.add)
            # ---- phase B: l2 streams in (chunked) once w1 is released ----
            l2 = wpool.tile([128, FC, D], F32R, tag="bigw")
            for kf in range(FC):
                nc.sync.dma_start(l2[:, kf, :], dram["el2T"][li][:, kf, :])
            rc2 = ln_stats_pair(x)
            for s in range(BL):
                sl = slice(s * S, (s + 1) * S)
                h2 = a4.tile([128, KC, S], F32R, tag="a4")
                ln_apply(rc2, x, s, h2)
                # ---- FFN ----
                f2 = [b1.tile([128, S], F32, tag="b1", name=f"f2_{_m}")
                      for _m in range(KC)]
                for kf in range(FC):
                    f1 = b2.tile([128, S], F32, tag="b2")
                    for k in range(KC):
                        nc.tensor.matmul(f1[:], l1[:, k, kf * 128:(kf + 1) * 128],
                                         h2[:, k, :], start=(k == 0),
                                         stop=(k == KC - 1))
                    rt = rtp.tile([128, S], F32R, tag="rt")
                    nc.scalar.activation(rt[:], f1[:], AF.Relu,
                                         bias=l1b[:, kf:kf + 1])
                    for m in range(KC):
                        nc.tensor.matmul(f2[m][:], l2[:, kf, m * 128:(m + 1) * 128],
                                         rt[:], start=(kf == 0), stop=(kf == FC - 1))
                for m in range(KC):
                    nc.vector.scalar_tensor_tensor(
                        x[:, m, sl], f2[m][:], l2b[:, m:m + 1], x[:, m, sl],
                        ALU.add, ALU.add)

        # ---------------- final encoder LN (in-place; affine folded) --------
        me = x
        rcf = ln_stats_pair(x)
        for s in range(BL):
            ln_apply(rcf, x, s,
                     x[:, :, s * S:(s + 1) * S])

        # ---------------- decoder layers ----------------
        for li in range(ld):
            dw = wpool.tile([128, KC, 3 * D], F32R, tag="bigw")
            nc.sync.dma_start(dw[:], dram["dinT"][li])
            dwb = bpool.tile([128, 12], F32, tag="w1b")
            nc.sync.dma_start(dwb[:], dram["dinb"][li])
            dvbrow = bpool.tile([1, D], F32R, tag="vbrow")
            nc.sync.dma_start(dvbrow[:], dram["dvb"][li])
            do = owpool.tile([128, KC, D], F32R, tag="ow")
            nc.sync.dma_start(do[:], dram["dowT"][li])
            dob = bpool.tile([128, 4], F32, tag="owb")
            nc.sync.dma_start(dob[:], dram["dowb"][li])
            m1 = wpool.tile([128, KC, M], F32R, tag="bigw")
            nc.sync.dma_start(m1[:], dram["dm1T"][li])
            m1b = bpool.tile([128, 16], F32, tag="l1b")
            nc.sync.dma_start(m1b[:], dram["dm1b"][li])
            m2b = bpool.tile([128, 4], F32, tag="l2b")
            nc.sync.dma_start(m2b[:], dram["dm2b"][li])

            vb_ps = b1.tile([128, D], F32, tag="b1")
            nc.tensor.matmul(vb_ps[:], ones_row[:], dvbrow[:], start=True, stop=True)
            vb_bc = vbp.tile([128, D], F32, tag="vb_bc")
            nc.scalar.activation(vb_bc[:], vb_ps[:], AF.Copy)

            # LN(p) -> q_ln ; Q projection (all samples at once, N=BL)
            q_ln = smalls.tile([128, KC, BL], F32R, tag="q_ln")
            ln(p, slice(None), BL, q_ln, slice(None))
            qps = b1.tile([128, KC, BL], F32, tag="b1")
            for m in range(KC):
                for k in range(KC):
                    nc.tensor.matmul(qps[:, m, :],
                                     c32(dw[:, k, m * 128:(m + 1) * 128]),
                                     c32(q_ln[:, k, :]), start=(k == 0),
                                     stop=(k == KC - 1))
            q_sb = smalls.tile([128, KC, BL], F32R, tag="q_sb")
            for m in range(KC):
                nc.scalar.activation(q_sb[:, m, :], qps[:, m, :], AF.Identity,
                                     bias=dwb[:, m:m + 1])
            o_d = smalls.tile([128, KC, BL], F32R, tag="o_d")
            for s in range(BL):
                sl = slice(s * S, (s + 1) * S)
                # K (feature-major) and V' (token-major) over morph_enc
                k_sb = a4.tile([128, KC, S], F32R, tag="a4")
                for m in range(KC):
                    ps = b1.tile([128, S], F32, tag="b1")
                    for k in range(KC):
                        nc.tensor.matmul(
                            ps[:], dw[:, k, D + m * 128:D + (m + 1) * 128],
                            me[:, k, sl], start=(k == 0), stop=(k == KC - 1))
                    nc.scalar.activation(k_sb[:, m, :], ps[:], AF.Identity,
                                         bias=dwb[:, 4 + m:5 + m])
                vloc = vp.tile([128, KC, 8, 65], F32R, tag="vloc")
                for t in range(KC):
                    nc.vector.tensor_copy(vloc[:, t, :, 64], ones8[:])
                for t in range(KC):
                    ps = b1.tile([128, S], F32, tag="b1")
                    for k in range(KC):
                        nc.tensor.matmul(
                            ps[:], me[:, k, s * S + t * 128:s * S + (t + 1) * 128],
                            dw[:, k, 2 * D:3 * D],
                            start=(k == 0), stop=(k == KC - 1))
                    nc.vector.tensor_tensor(
                        vloc[:, t, :, 0:64],
                        ps[:].rearrange("p (h d) -> p h d", h=H),
                        vb_bc[:].rearrange("p (h d) -> p h d", h=H), ALU.add)
                scp = b1.tile([128, KC, H], F32, tag="b1")
                for hh in range(H):
                    rows = slice(64 * (hh % 2), 64 * (hh % 2) + 64)
                    for c in range(KC):
                        nc.tensor.matmul(
                            scp[:, c, hh:hh + 1],
                            c32(k_sb[rows, hh // 2, c * 128:(c + 1) * 128]),
                            c32(q_sb[rows, hh // 2, s:s + 1]),
                            start=True, stop=True)
                at = smalls.tile([128, KC, H], F32R, tag="at_d")
                nc.scalar.activation(at[:], scp[:], AF.Exp,
                                     scale=float(1.0 / np.sqrt(DH)))
                ov = b2.tile([65, H], F32, tag="b2")
                for hh in range(H):
                    for c in range(KC):
                        nc.tensor.matmul(ov[:, hh:hh + 1], c32(vloc[:, c, hh, :]),
                                         c32(at[:, c, hh:hh + 1]),
                                         start=(c == 0), stop=(c == KC - 1))
                rec = scr.tile([1, H], F32R, tag="scr")
                nc.vector.reciprocal(rec[:], ov[64:65, :])
                rb = b2.tile([64, H], F32, tag="b2")
                nc.tensor.matmul(rb[:], c32(ones_row[:, 0:64]), c32(rec[:]),
                                 start=True, stop=True)
                rb_sb = scr.tile([64, H], F32, tag="scr")
                nc.scalar.activation(rb_sb[:], rb[:], AF.Copy)
                for hh in range(H):
                    rows = slice(64 * (hh % 2), 64 * (hh % 2) + 64)
                    nc.vector.tensor_tensor(o_d[rows, hh // 2, s:s + 1],
                                            ov[0:64, hh:hh + 1],
                                            rb_sb[:, hh:hh + 1], ALU.mult)
            # out-proj + residual into p
            ops = b1.tile([128, KC, BL], F32, tag="b1")
            for m in range(KC):
                for k in range(KC):
                    nc.tensor.matmul(ops[:, m, :],
                                     c32(do[:, k, m * 128:(m + 1) * 128]),
                                     c32(o_d[:, k, :]), start=(k == 0),
                                     stop=(k == KC - 1))
            for m in range(KC):
                nc.vector.scalar_tensor_tensor(
                    p[:, m, :], ops[:, m, :], dob[:, m:m + 1], p[:, m, :],
                    ALU.add, ALU.add)
            # FFN on p (m2 streams in chunked once dw releases its slot)
            m2 = wpool.tile([128, MC, D], F32R, tag="bigw")
            for kf in range(MC):
                nc.sync.dma_start(m2[:, kf, :], dram["dm2T"][li][:, kf, :])
            h2d = smalls.tile([128, KC, BL], F32R, tag="q_ln")
            ln(p, slice(None), BL, h2d, slice(None))
            mh = smalls.tile([128, MC, BL], F32R, tag="mh")
            for mm_ in range(MC):
                ps = b1.tile([128, BL], F32, tag="b1")
                for k in range(KC):
                    nc.tensor.matmul(ps[:], c32(m1[:, k, mm_ * 128:(mm_ + 1) * 128]),
                                     c32(h2d[:, k, :]), start=(k == 0),
                                     stop=(k == KC - 1))
                nc.scalar.activation(mh[:, mm_, :], ps[:], AF.Relu,
                                     bias=m1b[:, mm_:mm_ + 1])
            m2ps = b1.tile([128, KC, BL], F32, tag="b1")
            for m in range(KC):
                for kf in range(MC):
                    nc.tensor.matmul(m2ps[:, m, :],
                                     c32(m2[:, kf, m * 128:(m + 1) * 128]),
                                     c32(mh[:, kf, :]), start=(kf == 0),
                                     stop=(kf == MC - 1))
            for m in range(KC):
                nc.vector.scalar_tensor_tensor(
                    p[:, m, :], m2ps[:, m, :], m2b[:, m:m + 1], p[:, m, :],
                    ALU.add, ALU.add)

        # ---------------- head ----------------
        hw = smalls.tile([128, KC], F32R, tag="hw")
        hb = smalls.tile([1, 1], F32, tag="hb")
        nc.sync.dma_start(hw[:], dram["hwT"][:])
        nc.sync.dma_start(hb[:], dram["hb"][:])
        hg = smalls.tile([128, KC, BL], F32R, tag="q_ln")
        ln(p, slice(None), BL, hg, slice(None))
        hps = b2.tile([1, BL], F32, tag="b2")
        for k in range(KC):
            nc.tensor.matmul(hps[:], c32(hw[:, k:k + 1]), c32(hg[:, k, :]),
                             start=(k == 0), stop=(k == KC - 1))
        y_sb = smalls.tile([1, BL], F32, tag="y_sb")
        nc.scalar.activation(y_sb[:], hps[:], AF.Sigmoid, bias=hb[:])
        nc.sync.dma_start(y_dram[:], y_sb[:])


# ----------------------------------------------------------------------------
# entry point
# ----------------------------------------------------------------------------

_NC_CACHE = {}


def kernel(**inputs):
    return _run(inputs, LE, LD)


def _run(inputs, le, ld, trace=False):
    w = prep_weights(inputs, le, ld)
    morph = np.asarray(inputs["morph"], np.float32)
    pose = np.asarray(inputs["pose"], np.float32)
    in_maps = []
    for c in range(NCORES):
        im = dict(w)
        mo = morph[c * BL:(c + 1) * BL]                 # [BL, S, 3]
        im["morphT"] = np.ascontiguousarray(
            mo.transpose(2, 0, 1).reshape(3, T))
        im["poseT"] = np.ascontiguousarray(pose[c * BL:(c + 1) * BL].T)
        in_maps.append(im)

    if ("nc", le, ld) not in _NC_CACHE:
        _NC_CACHE[("nc", le, ld)] = build(le, ld)
    nc = _NC_CACHE[("nc", le, ld)]
    res = run_bass_kernel_spmd(nc, in_maps, core_ids=list(range(NCORES)),
                               trace=trace)
    out = np.zeros((B, 1), np.float32)
    for c in range(NCORES):
        out[c * BL:(c + 1) * BL, 0] = res.results[c]["y"][0]
    if trace:
        return out, res
    return out



# revision 26
# speedup vs baseline: 1.2238x; 1.2238x over previous
"""Trainium2 Bass kernel for nn_ReachabilityClassifierTransformer.

Data-parallel over batch: 16 samples / 8 cores = 2 samples per core.
Each core runs the full network (6-layer encoder + 4-layer decoder + head)
on its 2 samples. No collectives.

v2 changes over the original baseline:
  - fp16 operands everywhere on the hot path (PE same speed, FWL weight
    loads, 2x DVE rate, half the weight DMA).
  - reciprocal_approx_fast for softmax denominators (~5x faster than
    nc.vector.reciprocal, which was 410us of DVE time on the critical path).
  - LN rstd computed as Exp(-0.5*Ln(var+eps)) so the ACT engine stays on
    the natural_log_exp_and_others table set for the whole kernel (the
    sqrt<->exp set alternation cost ~2.7us per swap, 2 swaps/layer).
  - Sigmoid head computed via Exp + reciprocal (no sigmoid table load).
  - PSUM evacuations rebalanced: biased evacs moved to DVE tensor_scalar
    (one fused op) where ACT was the bottleneck.

Device layout conventions (per core):
  - Activations are kept FEATURE-MAJOR in SBUF: tile [128, KC, T] holds
    X.T, i.e. element [p, k, t] = X[t, k*128+p]. T = 2*512 tokens
    (sample-major concat).
  - All weights are pre-transposed on host to [in_feat, out_feat] and laid
    out as [128, KC_in, O] (partition = in-feature % 128), fp16.
  - matmul(out_psum[M,N], lhsT=[K,M], rhs=[K,N]) computes lhsT.T @ rhs with
    K on partitions.
  - Encoder stage-1 q,k output features are de-interleaved (even feats then
    odd feats) via host-side column permutation of in_proj, so RoPE becomes
    contiguous block ops; the roped result is in natural order again.
  - Softmax: scores are computed transposed (S.T = K_h @ Q_h.T per 128-row
    chunk), exp'd without max subtraction (|scores/8| < 1 for this model),
    and the denominator comes free from a ones-column appended to V.
"""
import numpy as np

import concourse.bass as bass
import concourse.mybir as mybir
import concourse.tile as tile
from concourse import bacc
from concourse.bass_utils import run_bass_kernel_spmd

AF = mybir.ActivationFunctionType
ALU = mybir.AluOpType
F32 = mybir.dt.float32
F32R = mybir.dt.float32r
F16 = mybir.dt.float16

B, S, D, FF, H, LE, LD, M = 16, 512, 512, 2048, 8, 6, 4, 2048
ROPE_BASE = 10000.0
LN_EPS = 1e-5
NCORES = 8
BL = B // NCORES          # 2 samples per core
T = BL * S                # 1024 tokens per core
KC = D // 128             # 4 feature chunks
FC = FF // 128            # 16
MC = M // 128             # 16
DH = D // H               # 64


# ----------------------------------------------------------------------------
# host-side helpers
# ----------------------------------------------------------------------------

def _chunked(wT, dt=np.float16):
    """[Din, O] -> [128, Din//128, O] contiguous."""
    Din, O = wT.shape
    return np.ascontiguousarray(
        wT.reshape(Din // 128, 128, O).transpose(1, 0, 2)).astype(dt)


def _bias_cols(b):
    """[O] -> [128, O//128]  (column per 128-chunk)."""
    O = b.shape[0]
    return np.ascontiguousarray(b.reshape(O // 128, 128).T).astype(np.float32)


_DEINT = np.concatenate([np.arange(0, D, 2), np.arange(1, D, 2)])  # de-interleave


def prep_weights(inp, le=LE, ld=LD):
    """Host-side weight prep -> dict of arrays shared by all cores."""
    out = {}
    g = {k: np.asarray(v, np.float32) for k, v in inp.items()}

    out["mpwT"] = np.ascontiguousarray(g["morph_proj_w"].T)        # [3, 512] f32
    out["mpb"] = _bias_cols(g["morph_proj_b"])                     # [128, 4]
    out["ppwT"] = np.ascontiguousarray(g["pose_proj_w"].T)         # [9, 512] f32
    out["ppb"] = _bias_cols(g["pose_proj_b"])

    # rope grids, de-interleaved frequency order: [128, 2, 512] f16
    freq = 1.0 / ROPE_BASE ** (np.arange(0, D, 2, dtype=np.float64) / D)
    ang = np.outer(np.arange(S, dtype=np.float64), freq)           # [512, 256]
    out["gridc"] = _chunked(np.cos(ang).T.astype(np.float32).reshape(256, S))
    out["grids"] = _chunked(np.sin(ang).T.astype(np.float32).reshape(256, S))

    e_w1, e_w1b, e_w2, e_w2b, e_vb = [], [], [], [], []
    e_ow, e_owb, e_l1, e_l1b, e_l2, e_l2b = [], [], [], [], [], []
    for i in range(le):
        w1 = g["enc_in_w"][i] * g["enc_n1_g"][i][None, :]          # fold n1 g
        b1 = g["enc_in_b"][i] + g["enc_in_w"][i] @ g["enc_n1_b"][i]
        # stage-1: de-interleave q,k output columns
        perm = np.concatenate([_DEINT, D + _DEINT, 2 * D + np.arange(D)])
        e_w1.append(_chunked(np.ascontiguousarray(w1[perm].T)))    # [128,4,1536]
        e_w1b.append(_bias_cols(b1[perm]))                         # [128,12]
        # stage-2 (natural order, raw weights - the faithful quirk)
        w2 = g["enc_in_w"][i][: 2 * D]                             # Wq;Wk
        e_w2.append(_chunked(np.ascontiguousarray(w2.T)))          # [128,4,1024]
        e_w2b.append(_bias_cols(g["enc_in_b"][i][: 2 * D]))        # [128,8]
        e_vb.append(g["enc_in_b"][i][2 * D:][None, :].astype(np.float16))  # [1,512]
        e_ow.append(_chunked(np.ascontiguousarray(g["enc_out_w"][i].T)))
        e_owb.append(_bias_cols(g["enc_out_b"][i]))
        l1 = g["enc_l1_w"][i] * g["enc_n2_g"][i][None, :]
        l1b = g["enc_l1_b"][i] + g["enc_l1_w"][i] @ g["enc_n2_b"][i]
        e_l1.append(_chunked(np.ascontiguousarray(l1.T)))          # [128,4,2048]
        e_l1b.append(_bias_cols(l1b))                              # [128,16]
        e_l2.append(_chunked(np.ascontiguousarray(g["enc_l2_w"][i].T)))
        e_l2b.append(_bias_cols(g["enc_l2_b"][i]))                 # [128,4]
    def _st(lst, shape, dt=np.float16):
        return np.stack(lst) if lst else np.zeros((0,) + shape, dt)
    out["ew1T"], out["ew1b"] = _st(e_w1, (128, KC, 3 * D)), _st(e_w1b, (128, 12), np.float32)
    out["ew2T"], out["ew2b"] = _st(e_w2, (128, KC, 2 * D)), _st(e_w2b, (128, 8), np.float32)
    out["evb"] = _st(e_vb, (1, D))
    out["eowT"], out["eowb"] = _st(e_ow, (128, KC, D)), _st(e_owb, (128, 4), np.float32)
    out["el1T"], out["el1b"] = _st(e_l1, (128, KC, FF)), _st(e_l1b, (128, 16), np.float32)
    out["el2T"], out["el2b"] = _st(e_l2, (128, FC, D)), _st(e_l2b, (128, 4), np.float32)

    d_in, d_inb, d_vb, d_ow, d_owb = [], [], [], [], []
    d_m1, d_m1b, d_m2, d_m2b = [], [], [], []
    for i in range(ld):
        w = g["dec_in_w"][i].copy()
        b = g["dec_in_b"][i].copy()
        w[:D] = w[:D] * g["dec_n1_g"][i][None, :]                  # Wq <- dec_n1
        b[:D] = b[:D] + g["dec_in_w"][i][:D] @ g["dec_n1_b"][i]
        w[D:] = w[D:] * g["enc_final_g"][None, :]                  # Wk,Wv <- enc_final
        b[D:] = b[D:] + g["dec_in_w"][i][D:] @ g["enc_final_b"]
        d_in.append(_chunked(np.ascontiguousarray(w.T)))           # [128,4,1536]
        d_inb.append(_bias_cols(b))
        d_vb.append(b[2 * D:][None, :].astype(np.float16))         # [1,512]
        d_ow.append(_chunked(np.ascontiguousarray(g["dec_out_w"][i].T), np.float32))
        d_owb.append(_bias_cols(g["dec_out_b"][i]))
        m1 = g["dec_m1_w"][i] * g["dec_n2_g"][i][None, :]
        m1b = g["dec_m1_b"][i] + g["dec_m1_w"][i] @ g["dec_n2_b"][i]
        d_m1.append(_chunked(np.ascontiguousarray(m1.T)))          # [128,4,2048]
        d_m1b.append(_bias_cols(m1b))
        d_m2.append(_chunked(np.ascontiguousarray(g["dec_m2_w"][i].T)))
        d_m2b.append(_bias_cols(g["dec_m2_b"][i]))
    out["dinT"], out["dinb"] = _st(d_in, (128, KC, 3 * D)), _st(d_inb, (128, 12), np.float32)
    out["dvb"] = _st(d_vb, (1, D))
    out["dowT"], out["dowb"] = _st(d_ow, (128, KC, D), np.float32), _st(d_owb, (128, 4), np.float32)
    out["dm1T"], out["dm1b"] = _st(d_m1, (128, KC, M)), _st(d_m1b, (128, 16), np.float32)
    out["dm2T"], out["dm2b"] = _st(d_m2, (128, MC, D)), _st(d_m2b, (128, 4), np.float32)

    hw = (g["head_w"] * g["head_g"][None, :])[0]                   # [512]
    hw_pad = np.zeros((128, D // 128, 8), np.float16)
    hw_pad[:, :, 0] = _bias_cols(hw)
    out["hwT"] = hw_pad                                            # [128, 4, 8] f16
    out["hbn"] = -(g["head_bias"] + g["head_w"] @ g["head_b"]).reshape(1, 1)
    return out


# ----------------------------------------------------------------------------
# device program
# ----------------------------------------------------------------------------

def build(le=LE, ld=LD):
    nc = bacc.Bacc(None, target_bir_lowering=False)

    dram = {}

    def din(name, shape, dt=F16):
        dram[name] = nc.dram_tensor(name, list(shape), dt, kind="ExternalInput")
        return dram[name]

    # shared weights
    din("mpwT", [3, 512], F32R); din("mpb", [128, 4], F32)
    din("ppwT", [9, 512], F32R); din("ppb", [128, 4], F32)
    din("gridc", [128, 2, S]); din("grids", [128, 2, S])
    din("ew1T", [le, 128, KC, 3 * D]); din("ew1b", [le, 128, 12], F32)
    din("ew2T", [le, 128, KC, 2 * D]); din("ew2b", [le, 128, 8], F32)
    din("evb", [le, 1, D])
    din("eowT", [le, 128, KC, D]); din("eowb", [le, 128, 4], F32)
    din("el1T", [le, 128, KC, FF]); din("el1b", [le, 128, 16], F32)
    din("el2T", [le, 128, FC, D]); din("el2b", [le, 128, 4], F32)
    din("dinT", [ld, 128, KC, 3 * D]); din("dinb", [ld, 128, 12], F32)
    din("dvb", [ld, 1, D])
    din("dowT", [ld, 128, KC, D], F32R); din("dowb", [ld, 128, 4], F32)
    din("dm1T", [ld, 128, KC, M]); din("dm1b", [ld, 128, 16], F32)
    din("dm2T", [ld, 128, MC, D]); din("dm2b", [ld, 128, 4], F32)
    din("hwT", [128, KC, 8]); din("hbn", [1, 1], F32)
    # per-core inputs
    din("morphT", [3, T], F32R)
    din("poseT", [9, BL], F32R)
    y = nc.dram_tensor("y", [1, BL], F32, kind="ExternalOutput")

    with tile.TileContext(nc) as tc:
        _build_body(nc, tc, dram, y, le, ld)
    nc.compile()
    return nc


def _build_body(nc, tc, dram, y_dram, le, ld):
    import contextlib
    ctx = contextlib.ExitStack()
    with ctx:
        ctx.enter_context(nc.allow_low_precision(
            reason="fp16 operands / approx reciprocal are intentional"))
        persist = ctx.enter_context(tc.tile_pool(name="persist", bufs=1))
        wpool = ctx.enter_context(tc.tile_pool(name="wpool", bufs=2))
        w2pool = ctx.enter_context(tc.tile_pool(name="w2pool", bufs=1))
        owpool = ctx.enter_context(tc.tile_pool(name="owpool", bufs=1))
        bpool = ctx.enter_context(tc.tile_pool(name="bpool", bufs=2))
        a4 = ctx.enter_context(tc.tile_pool(name="a4", bufs=3))
        a8 = ctx.enter_context(tc.tile_pool(name="a8", bufs=2))
        vp = ctx.enter_context(tc.tile_pool(name="vp", bufs=1))
        vpd = ctx.enter_context(tc.tile_pool(name="vpd", bufs=1))
        rtp = ctx.enter_context(tc.tile_pool(name="rtp", bufs=3))
        vbp = ctx.enter_context(tc.tile_pool(name="vbp", bufs=1))
        scr = ctx.enter_context(tc.tile_pool(name="scr", bufs=2))
        scrrc = ctx.enter_context(tc.tile_pool(name="scrrc", bufs=1))
        smalls = ctx.enter_context(tc.tile_pool(name="smalls", bufs=2))
        b1 = ctx.enter_context(tc.tile_pool(name="b1", bufs=4, space="PSUM"))
        b2 = ctx.enter_context(tc.tile_pool(name="b2", bufs=2, space="PSUM"))
        qk2p, atp = a8, a4  # share slots/tags

        # ---------------- persistent tiles ----------------
        x = persist.tile([128, KC, T], F16)           # residual stream (X.T)
        vloc_d = persist.tile([128, KC, 8, 65], F32R)  # decoder V (no pool)
        gridc = persist.tile([128, 2, S], F16)
        grids = persist.tile([128, 2, S], F16)
        ones128h = persist.tile([128, 1], F16)
        ones_row = persist.tile([1, 128], F32R)
        ones_rowh = persist.tile([1, 128], F16)
        eps_t = persist.tile([1, 1], F32)
        p = persist.tile([128, KC, BL], F16)          # decoder latent p.T
        nc.sync.dma_start(gridc[:], dram["gridc"][:])
        nc.sync.dma_start(grids[:], dram["grids"][:])
        ones8h = persist.tile([128, 8], F16)
        ones8r = persist.tile([128, 8], F32R)
        stage_f32 = rtp.tile([128, 128], F32, tag="rt")
        nc.vector.memset(stage_f32[:], 1.0)
        nc.vector.tensor_copy(ones128h[:], stage_f32[:, 0:1])
        nc.vector.tensor_copy(ones_row[:], stage_f32[0:1, :])
        nc.vector.tensor_copy(ones_rowh[:], stage_f32[0:1, :])
        nc.vector.tensor_copy(ones8h[:], stage_f32[:, 0:8])
        nc.vector.tensor_copy(ones8r[:], stage_f32[:, 0:8])
        nc.vector.memset(eps_t[:], LN_EPS)

        def c32(ap):
            return ap.bitcast(F32)

        def rstd_from_var(t2):
            """t2 (f32 [1,n]) <- 1/sqrt(t2+eps), via exp(-0.5*ln(t2+eps)).

            Keeps ACT on the natural_log_exp table set (no sqrt set swap)."""
            nc.scalar.activation(t2, t2, AF.Ln, bias=eps_t[:])
            nc.scalar.activation(t2, t2, AF.Exp, scale=-0.5)

        def ln(x_tile, sl, n_tok, h_out, out_sl):
            """h_out[:, :, out_sl] = LayerNorm_features(x_tile[:, :, sl])."""
            sq = a4.tile([128, KC, n_tok], F16, tag="a4")
            for k in range(KC):
                nc.vector.tensor_tensor(sq[:, k, :], x_tile[:, k, sl],
                                        x_tile[:, k, sl], ALU.mult)
            sum_ps = b2.tile([1, n_tok], F32, tag="b2")
            sq_ps = b2.tile([1, n_tok], F32, tag="b2")
            for k in range(KC):
                nc.tensor.matmul(sum_ps[:], ones128h[:], x_tile[:, k, sl],
                                 start=(k == 0), stop=(k == KC - 1))
            for k in range(KC):
                nc.tensor.matmul(sq_ps[:], ones128h[:], sq[:, k, :],
                                 start=(k == 0), stop=(k == KC - 1))
            ms = scr.tile([1, n_tok], F32, tag="scr")
            t2 = scr.tile([1, n_tok], F32, tag="scr")
            rc = scrrc.tile([1, 2, n_tok], F32R, tag="scr_rc")
            nc.scalar.activation(ms[:], sum_ps[:], AF.Copy, scale=1.0 / D)
            nc.vector.tensor_tensor(t2[:], ms[:], ms[:], ALU.mult)      # m^2
            nc.vector.scalar_tensor_tensor(
                t2[:], sq_ps[:], 1.0 / D, t2[:], ALU.mult, ALU.subtract)
            rstd_from_var(t2[:])
            nc.vector.tensor_copy(rc[:, 0, :], t2[:])
            nc.vector.scalar_tensor_tensor(
                rc[:, 1, :], ms[:], -1.0, rc[:, 0, :], ALU.mult, ALU.mult)  # c
            # fp32r misbehaves at tiny moving dims -- bitcast to plain f32
            # for n_tok < 256 (baseline did this via cv()).
            cv = (lambda ap: ap) if n_tok >= 256 else c32
            r_ps = b1.tile([128, n_tok], F32, tag="b1")
            c_ps = b1.tile([128, n_tok], F32, tag="b1")
            nc.tensor.matmul(r_ps[:], cv(ones_row[:]), cv(rc[:, 0, :]),
                             start=True, stop=True)
            nc.tensor.matmul(c_ps[:], cv(ones_row[:]), cv(rc[:, 1, :]),
                             start=True, stop=True)
            for k in range(KC):
                nc.vector.tensor_tensor(h_out[:, k, out_sl], x_tile[:, k, sl],
                                        r_ps[:], ALU.mult)
                nc.vector.tensor_tensor(h_out[:, k, out_sl], h_out[:, k, out_sl],
                                        c_ps[:], ALU.add)

        def ln_stats_pair(x_tile):
            """One LN chain for both samples: returns rc [1, 2, T] f32r."""
            sq = a8.tile([128, KC, T], F16, tag="a8")
            for k in range(KC):
                nc.vector.tensor_tensor(sq[:, k, :], x_tile[:, k, :],
                                        x_tile[:, k, :], ALU.mult)
            sum_ps = b2.tile([1, T], F32, tag="b2")
            sq_ps = b2.tile([1, T], F32, tag="b2")
            for nh in range(BL):
                nsl = slice(nh * S, (nh + 1) * S)
                for k in range(KC):
                    nc.tensor.matmul(sum_ps[:, nsl], ones128h[:],
                                     x_tile[:, k, nsl],
                                     start=(k == 0), stop=(k == KC - 1))
                for k in range(KC):
                    nc.tensor.matmul(sq_ps[:, nsl], ones128h[:], sq[:, k, nsl],
                                     start=(k == 0), stop=(k == KC - 1))
            rc = scrrc.tile([1, 2, T], F32R, tag="scr_rcT")
            ms = rc[:, 1, :]                      # mean parked in the c slot
            t2 = scrrc.tile([1, T], F32, tag="scrT2")
            nc.scalar.activation(ms, sum_ps[:], AF.Copy, scale=1.0 / D)
            nc.vector.tensor_tensor(t2[:], ms, ms, ALU.mult)
            nc.vector.scalar_tensor_tensor(
                t2[:], sq_ps[:], 1.0 / D, t2[:], ALU.mult, ALU.subtract)
            rstd_from_var(t2[:])
            nc.vector.scalar_tensor_tensor(
                rc[:, 1, :], ms, -1.0, t2[:], ALU.mult, ALU.mult)
            nc.vector.tensor_copy(rc[:, 0, :], t2[:])
            return rc

        def ln_apply(rc, x_tile, s, h_out):
            sl = slice(s * S, (s + 1) * S)
            r_ps = b1.tile([128, S], F32, tag="b1")
            c_ps = b1.tile([128, S], F32, tag="b1")
            nc.tensor.matmul(r_ps[:], ones_row[:], rc[:, 0, sl],
                             start=True, stop=True)
            nc.tensor.matmul(c_ps[:], ones_row[:], rc[:, 1, sl],
                             start=True, stop=True)
            for k in range(KC):
                nc.vector.tensor_tensor(h_out[:, k, :], x_tile[:, k, sl],
                                        r_ps[:], ALU.mult)
                nc.vector.tensor_tensor(h_out[:, k, :], h_out[:, k, :],
                                        c_ps[:], ALU.add)

        # ---------------- morph projection -> x ----------------
        morpht = a8.tile([3, T], F32R, tag="a8")
        nc.sync.dma_start(morpht[:], dram["morphT"][:])
        mpw = rtp.tile([3, 512], F32R, tag="rt")
        mpb = rtp.tile([128, 4], F32, tag="rt")
        nc.sync.dma_start(mpw[:], dram["mpwT"][:])
        nc.sync.dma_start(mpb[:], dram["mpb"][:])
        for m in range(KC):
            for s in range(BL):
                ps = b1.tile([128, S], F32, tag="b1")
                nc.tensor.matmul(ps[:], mpw[:, m * 128:(m + 1) * 128],
                                 morpht[:, s * S:(s + 1) * S], start=True, stop=True)
                nc.scalar.activation(x[:, m, s * S:(s + 1) * S], ps[:], AF.Relu,
                                     bias=mpb[:, m:m + 1])

        # ---------------- pose projection -> p ----------------
        poset = rtp.tile([9, BL], F32R, tag="rt")
        ppw = rtp.tile([9, 512], F32R, tag="rt")
        ppb = rtp.tile([128, 4], F32, tag="rt")
        nc.sync.dma_start(poset[:], dram["poseT"][:])
        nc.sync.dma_start(ppw[:], dram["ppwT"][:])
        nc.sync.dma_start(ppb[:], dram["ppb"][:])
        pps = b1.tile([128, KC, BL], F32, tag="b1")
        for m in range(KC):
            nc.tensor.matmul(pps[:, m, :], c32(ppw[:, m * 128:(m + 1) * 128]),
                             c32(poset[:]), start=True, stop=True)
        for m in range(KC):
            nc.scalar.activation(p[:, m, :], pps[:, m, :], AF.Relu,
                                 bias=ppb[:, m:m + 1])

        # ---------------- encoder layers ----------------
        for li in range(le):
            w1 = wpool.tile([128, KC, 3 * D], F16, tag="bigw")
            nc.sync.dma_start(w1[:], dram["ew1T"][li])
            w1b = bpool.tile([128, 12], F32, tag="w1b")
            nc.sync.dma_start(w1b[:], dram["ew1b"][li])
            w2 = w2pool.tile([128, KC, 2 * D], F16, tag="w2")
            nc.sync.dma_start(w2[:], dram["ew2T"][li])
            w2b = bpool.tile([128, 8], F32, tag="w2b")
            nc.sync.dma_start(w2b[:], dram["ew2b"][li])
            vbrow = bpool.tile([1, D], F16, tag="vbrow")
            nc.sync.dma_start(vbrow[:], dram["evb"][li])
            ow = owpool.tile([128, KC, D], F16, tag="ow")
            nc.sync.dma_start(ow[:], dram["eowT"][li])
            owb = bpool.tile([128, 4], F32, tag="owb")
            nc.sync.dma_start(owb[:], dram["eowb"][li])
            l1 = wpool.tile([128, KC, FF], F16, tag="bigw")
            nc.sync.dma_start(l1[:], dram["el1T"][li])
            l1b = bpool.tile([128, 16], F32, tag="l1b")
            nc.sync.dma_start(l1b[:], dram["el1b"][li])
            l2b = bpool.tile([128, 4], F32, tag="l2b")
            nc.sync.dma_start(l2b[:], dram["el2b"][li])

            # v-bias broadcast [128, 512] (token-major V bias), once per layer
            vb_ps = b1.tile([128, D], F32, tag="b1")
            nc.tensor.matmul(vb_ps[:], ones_rowh[:], vbrow[:], start=True, stop=True)
            vb_bc = vbp.tile([128, D], F32, tag="vb_bc")
            nc.scalar.activation(vb_bc[:], vb_ps[:], AF.Copy)

            rc1 = ln_stats_pair(x)
            for s in range(BL):
                sl = slice(s * S, (s + 1) * S)
                h = a4.tile([128, KC, S], F16, tag="a4")
                ln_apply(rc1, x, s, h)
                # ---- stage 1: q,k (permuted) + v ----
                qkv1 = a8.tile([128, 8, S], F16, tag="a8")
                v1 = a4.tile([128, KC, S], F16, tag="a4")
                for m in range(12):
                    ps = b1.tile([128, S], F32, tag="b1")
                    for k in range(KC):
                        nc.tensor.matmul(ps[:], w1[:, k, m * 128:(m + 1) * 128],
                                         h[:, k, :], start=(k == 0), stop=(k == KC - 1))
                    dest = qkv1[:, m, :] if m < 8 else v1[:, m - 8, :]
                    nc.vector.tensor_scalar_add(dest, ps[:], w1b[:, m:m + 1])
                # ---- rope: qkv1 (chunks 0-7) -> qkr (natural order) ----
                qkr = a8.tile([128, 8, S], F16, tag="a8")
                for half in (0, 4):
                    for c in range(2):
                        e = qkv1[:, half + c, :]
                        o = qkv1[:, half + 2 + c, :]
                        r1 = qkr[:, half + c, :]
                        r2 = qkr[:, half + 2 + c, :]
                        t1 = rtp.tile([128, S], F16, tag="rt")
                        nc.vector.tensor_tensor(r1, e, gridc[:, c, :], ALU.mult)
                        nc.vector.tensor_tensor(t1[:], o, grids[:, c, :], ALU.mult)
                        nc.vector.tensor_tensor(r1, r1, t1[:], ALU.subtract)
                        t2 = rtp.tile([128, S], F16, tag="rt")
                        nc.vector.tensor_tensor(r2, e, grids[:, c, :], ALU.mult)
                        nc.vector.tensor_tensor(t2[:], o, gridc[:, c, :], ALU.mult)
                        nc.vector.tensor_tensor(r2, r2, t2[:], ALU.add)
                # ---- stage 2: Q,K ----
                qk2 = qk2p.tile([128, 8, S], F16, tag="a8")
                for m in range(8):
                    ps = b1.tile([128, S], F32, tag="b1")
                    base = 0 if m < 4 else 4
                    for k in range(KC):
                        nc.tensor.matmul(ps[:], w2[:, k, m * 128:(m + 1) * 128],
                                         qkr[:, base + k, :],
                                         start=(k == 0), stop=(k == KC - 1))
                    nc.scalar.activation(qk2[:, m, :], ps[:], AF.Identity,
                                         bias=w2b[:, m:m + 1])
                # ---- stage 2: V (token-major, with ones column per head) ----
                # last dim padded 65->72 so per-head slices stay 16B-aligned
                # in fp16 (65*2B = 130B stride broke odd-head matmul reads)
                vloc = vp.tile([128, KC, 8, 72], F16, tag="vloc")
                for t in range(KC):
                    nc.vector.tensor_copy(vloc[:, t, :, 64], ones8h[:])
                for t in range(KC):
                    ps = b1.tile([128, S], F32, tag="b1")
                    for k in range(KC):
                        nc.tensor.matmul(
                            ps[:], v1[:, k, t * 128:(t + 1) * 128],
                            w1[:, k, 2 * D:3 * D],
                            start=(k == 0), stop=(k == KC - 1))
                    nc.vector.tensor_tensor(
                        vloc[:, t, :, 0:64],
                        ps[:].rearrange("p (h d) -> p h d", h=H),
                        vb_bc[:].rearrange("p (h d) -> p h d", h=H), ALU.add)
                # ---- attention heads (paired: exp(h+1) hides under A@V(h)) ----
                o_t = a4.tile([128, KC, S], F16, tag="a4")
                for h0 in range(0, H, 2):
                    ats = {}
                    for hh in (h0, h0 + 1):
                        rows = slice(64 * (hh % 2), 64 * (hh % 2) + 64)
                        at = atp.tile([128, KC, S], F16, tag="a4")
                        for c in range(KC):
                            scp = b1.tile([128, S], F32, tag="b1")
                            nc.tensor.matmul(
                                scp[:],
                                qk2[rows, 4 + hh // 2, c * 128:(c + 1) * 128],
                                qk2[rows, hh // 2, :], start=True, stop=True)
                            nc.scalar.activation(at[:, c, :], scp[:], AF.Exp,
                                                 scale=float(1.0 / np.sqrt(DH)))
                        ats[hh] = at
                    for hh in (h0, h0 + 1):
                        rows = slice(64 * (hh % 2), 64 * (hh % 2) + 64)
                        at = ats[hh]
                        ov = b2.tile([65, S], F32, tag="b2")
                        for c in range(KC):
                            nc.tensor.matmul(ov[:], vloc[:, c, hh, 0:65],
                                             at[:, c, :],
                                             start=(c == 0), stop=(c == KC - 1))
                        # custom DVE recip mishandles nonzero psum base
                        # partition: stage the denominator row to SBUF p0
                        # via ACT Copy (regular op) first.
                        t_den = scr.tile([1, S], F32, tag="scr")
                        nc.scalar.activation(t_den[:], ov[64:65, :], AF.Copy)
                        rec = scr.tile([1, S], F32, tag="scr")
                        nc.vector.reciprocal_approx_fast(rec[:], t_den[:])
                        rec_h = scr.tile([1, S], F16, tag="scr")
                        nc.vector.tensor_copy(rec_h[:], rec[:])
                        rb = b2.tile([64, S], F32, tag="b2")
                        nc.tensor.matmul(rb[:], ones_rowh[:, 0:64], rec_h[:],
                                         start=True, stop=True)
                        rb_sb = scr.tile([64, S], F16, tag="scr")
                        nc.scalar.activation(rb_sb[:], rb[:], AF.Copy)
                        nc.vector.tensor_tensor(o_t[rows, hh // 2, :],
                                                ov[0:64, :], rb_sb[:], ALU.mult)
                # ---- out-proj + residual ----
                for m in range(KC):
                    ps = b1.tile([128, S], F32, tag="b1")
                    for k in range(KC):
                        nc.tensor.matmul(ps[:], ow[:, k, m * 128:(m + 1) * 128],
                                         o_t[:, k, :], start=(k == 0),
                                         stop=(k == KC - 1))
                    nc.vector.scalar_tensor_tensor(
                        x[:, m, sl], ps[:], owb[:, m:m + 1], x[:, m, sl],
                        ALU.add, ALU.add)
            # ---- phase B: l2 streams in (chunked) once w1 is released ----
            l2 = wpool.tile([128, FC, D], F16, tag="bigw")
            for kf in range(FC):
                nc.sync.dma_start(l2[:, kf, :], dram["el2T"][li][:, kf, :])
            rc2 = ln_stats_pair(x)
            for s in range(BL):
                sl = slice(s * S, (s + 1) * S)
                h2 = a4.tile([128, KC, S], F16, tag="a4")
                ln_apply(rc2, x, s, h2)
                # ---- FFN ----
                f2 = [b1.tile([128, S], F32, tag="b1", name=f"f2_{_m}")
                      for _m in range(KC)]
                for kf in range(FC):
                    f1 = b2.tile([128, S], F32, tag="b2")
                    for k in range(KC):
                        nc.tensor.matmul(f1[:], l1[:, k, kf * 128:(kf + 1) * 128],
                                         h2[:, k, :], start=(k == 0),
                                         stop=(k == KC - 1))
                    rt = rtp.tile([128, S], F16, tag="rt")
                    nc.vector.tensor_scalar(rt[:], f1[:], l1b[:, kf:kf + 1], 0.0,
                                            ALU.add, ALU.max)
                    for m in range(KC):
                        nc.tensor.matmul(f2[m][:], l2[:, kf, m * 128:(m + 1) * 128],
                                         rt[:], start=(kf == 0), stop=(kf == FC - 1))
                for m in range(KC):
                    nc.vector.scalar_tensor_tensor(
                        x[:, m, sl], f2[m][:], l2b[:, m:m + 1], x[:, m, sl],
                        ALU.add, ALU.add)

        # ---------------- final encoder LN (in-place; affine folded) --------
        me = x
        rcf = ln_stats_pair(x)
        for s in range(BL):
            ln_apply(rcf, x, s,
                     x[:, :, s * S:(s + 1) * S])

        # ---------------- decoder layers ----------------
        for li in range(ld):
            dw = wpool.tile([128, KC, 3 * D], F16, tag="bigw")
            nc.sync.dma_start(dw[:], dram["dinT"][li])
            dwb = bpool.tile([128, 12], F32, tag="w1b")
            nc.sync.dma_start(dwb[:], dram["dinb"][li])
            dvbrow = bpool.tile([1, D], F16, tag="vbrow")
            nc.sync.dma_start(dvbrow[:], dram["dvb"][li])
            do = owpool.tile([128, KC, D], F32R, tag="ow")
            nc.sync.dma_start(do[:], dram["dowT"][li])
            dob = bpool.tile([128, 4], F32, tag="owb")
            nc.sync.dma_start(dob[:], dram["dowb"][li])
            m1 = wpool.tile([128, KC, M], F16, tag="bigw")
            nc.sync.dma_start(m1[:], dram["dm1T"][li])
            m1b = bpool.tile([128, 16], F32, tag="l1b")
            nc.sync.dma_start(m1b[:], dram["dm1b"][li])
            m2b = bpool.tile([128, 4], F32, tag="l2b")
            nc.sync.dma_start(m2b[:], dram["dm2b"][li])

            vb_ps = b1.tile([128, D], F32, tag="b1")
            nc.tensor.matmul(vb_ps[:], ones_rowh[:], dvbrow[:], start=True, stop=True)
            vb_bc = vbp.tile([128, D], F32, tag="vb_bc")
            nc.scalar.activation(vb_bc[:], vb_ps[:], AF.Copy)

            # LN(p) -> q_ln ; Q projection (all samples at once, N=BL)
            q_ln = smalls.tile([128, KC, 8], F16, tag="q_ln")
            ln(p, slice(None), BL, q_ln, slice(0, BL))
            qps = b1.tile([128, KC, BL], F32, tag="b1")
            for m in range(KC):
                for k in range(KC):
                    nc.tensor.matmul(qps[:, m, :],
                                     dw[:, k, m * 128:(m + 1) * 128],
                                     q_ln[:, k, 0:BL], start=(k == 0),
                                     stop=(k == KC - 1))
            q_sb = smalls.tile([128, KC, BL], F32R, tag="q_sb")
            for m in range(KC):
                nc.scalar.activation(q_sb[:, m, :], qps[:, m, :], AF.Identity,
                                     bias=dwb[:, m:m + 1])
            o_d = smalls.tile([128, KC, BL], F32R, tag="o_d")
            for s in range(BL):
                sl = slice(s * S, (s + 1) * S)
                # K (feature-major) and V' (token-major) over morph_enc
                k_sb = a4.tile([128, KC, S], F32R, tag="a4")
                for m in range(KC):
                    ps = b1.tile([128, S], F32, tag="b1")
                    for k in range(KC):
                        nc.tensor.matmul(
                            ps[:], dw[:, k, D + m * 128:D + (m + 1) * 128],
                            me[:, k, sl], start=(k == 0), stop=(k == KC - 1))
                    nc.scalar.activation(k_sb[:, m, :], ps[:], AF.Identity,
                                         bias=dwb[:, 4 + m:5 + m])
                vloc = vloc_d
                for t in range(KC):
                    nc.vector.tensor_copy(vloc[:, t, :, 64], ones8r[:])
                for t in range(KC):
                    ps = b1.tile([128, S], F32, tag="b1")
                    for k in range(KC):
                        nc.tensor.matmul(
                            ps[:], me[:, k, s * S + t * 128:s * S + (t + 1) * 128],
                            dw[:, k, 2 * D:3 * D],
                            start=(k == 0), stop=(k == KC - 1))
                    nc.vector.tensor_tensor(
                        vloc[:, t, :, 0:64],
                        ps[:].rearrange("p (h d) -> p h d", h=H),
                        vb_bc[:].rearrange("p (h d) -> p h d", h=H), ALU.add)
                scp = b1.tile([128, KC, H], F32, tag="b1")
                for hh in range(H):
                    rows = slice(64 * (hh % 2), 64 * (hh % 2) + 64)
                    for c in range(KC):
                        nc.tensor.matmul(
                            scp[:, c, hh:hh + 1],
                            c32(k_sb[rows, hh // 2, c * 128:(c + 1) * 128]),
                            c32(q_sb[rows, hh // 2, s:s + 1]),
                            start=True, stop=True)
                at = smalls.tile([128, KC, H], F32R, tag="at_d")
                nc.scalar.activation(at[:], scp[:], AF.Exp,
                                     scale=float(1.0 / np.sqrt(DH)))
                ov = b2.tile([65, H], F32, tag="b2")
                for hh in range(H):
                    for c in range(KC):
                        nc.tensor.matmul(ov[:, hh:hh + 1], c32(vloc[:, c, hh, :]),
                                         c32(at[:, c, hh:hh + 1]),
                                         start=(c == 0), stop=(c == KC - 1))
                rec = scr.tile([1, H], F32R, tag="scr")
                nc.vector.reciprocal(rec[:], ov[64:65, :])
                rb = b2.tile([64, H], F32, tag="b2")
                nc.tensor.matmul(rb[:], c32(ones_row[:, 0:64]), c32(rec[:]),
                                 start=True, stop=True)
                rb_sb = scr.tile([64, H], F32, tag="scr")
                nc.scalar.activation(rb_sb[:], rb[:], AF.Copy)
                for hh in range(H):
                    rows = slice(64 * (hh % 2), 64 * (hh % 2) + 64)
                    nc.vector.tensor_tensor(o_d[rows, hh // 2, s:s + 1],
                                            ov[0:64, hh:hh + 1],
                                            rb_sb[:, hh:hh + 1], ALU.mult)
            # out-proj + residual into p
            ops = b1.tile([128, KC, BL], F32, tag="b1")
            for m in range(KC):
                for k in range(KC):
                    nc.tensor.matmul(ops[:, m, :],
                                     c32(do[:, k, m * 128:(m + 1) * 128]),
                                     c32(o_d[:, k, :]), start=(k == 0),
                                     stop=(k == KC - 1))
            for m in range(KC):
                nc.vector.scalar_tensor_tensor(
                    p[:, m, :], ops[:, m, :], dob[:, m:m + 1], p[:, m, :],
                    ALU.add, ALU.add)
            # FFN on p (m2 streams in chunked once dw releases its slot)
            m2 = wpool.tile([128, MC, D], F16, tag="bigw")
            for kf in range(MC):
                nc.sync.dma_start(m2[:, kf, :], dram["dm2T"][li][:, kf, :])
            h2d = smalls.tile([128, KC, 8], F16, tag="q_ln")
            ln(p, slice(None), BL, h2d, slice(0, BL))
            mh = smalls.tile([128, MC, 8], F16, tag="mh")
            for mm_ in range(MC):
                ps = b1.tile([128, BL], F32, tag="b1")
                for k in range(KC):
                    nc.tensor.matmul(ps[:], m1[:, k, mm_ * 128:(mm_ + 1) * 128],
                                     h2d[:, k, 0:BL], start=(k == 0),
                                     stop=(k == KC - 1))
                nc.vector.tensor_scalar(mh[:, mm_, 0:BL], ps[:],
                                        m1b[:, mm_:mm_ + 1],
                                        0.0, ALU.add, ALU.max)
            m2ps = b1.tile([128, KC, BL], F32, tag="b1")
            for m in range(KC):
                for kf in range(MC):
                    nc.tensor.matmul(m2ps[:, m, :],
                                     m2[:, kf, m * 128:(m + 1) * 128],
                                     mh[:, kf, 0:BL], start=(kf == 0),
                                     stop=(kf == MC - 1))
            for m in range(KC):
                nc.vector.scalar_tensor_tensor(
                    p[:, m, :], m2ps[:, m, :], m2b[:, m:m + 1], p[:, m, :],
                    ALU.add, ALU.add)

        # ---------------- head (sigmoid via exp + recip; no table swap) -----
        hw = smalls.tile([128, KC, 8], F16, tag="hw")
        hbn = smalls.tile([1, 1], F32, tag="hb")
        nc.sync.dma_start(hw[:], dram["hwT"][:])
        nc.sync.dma_start(hbn[:], dram["hbn"][:])
        hg = smalls.tile([128, KC, 8], F16, tag="q_ln")
        ln(p, slice(None), BL, hg, slice(0, BL))
        hps = b2.tile([1, BL], F32, tag="b2")
        for k in range(KC):
            nc.tensor.matmul(hps[:], hw[:, k, 0:1], hg[:, k, 0:BL],
                             start=(k == 0), stop=(k == KC - 1))
        e_t = smalls.tile([1, BL], F32, tag="y_e")
        nc.scalar.activation(e_t[:], hps[:], AF.Exp, scale=-1.0, bias=hbn[:])
        nc.vector.tensor_scalar_add(e_t[:], e_t[:], 1.0)
        y_sb = smalls.tile([1, BL], F32, tag="y_sb")
        nc.vector.reciprocal_approx_fast(y_sb[:], e_t[:])
        nc.sync.dma_start(y_dram[:], y_sb[:])


# ----------------------------------------------------------------------------
# entry point
# ----------------------------------------------------------------------------

_NC_CACHE = {}


def kernel(**inputs):
    return _run(inputs, LE, LD)


def _run(inputs, le, ld, trace=False):
    w = prep_weights(inputs, le, ld)
    morph = np.asarray(inputs["morph"], np.float32)
    pose = np.asarray(inputs["pose"], np.float32)
    in_maps = []
    for c in range(NCORES):
        im = dict(w)
        mo = morph[c * BL:(c + 1) * BL]                 # [BL, S, 3]
        im["morphT"] = np.ascontiguousarray(
            mo.transpose(2, 0, 1).reshape(3, T))
        im["poseT"] = np.ascontiguousarray(pose[c * BL:(c + 1) * BL].T)
        in_maps.append(im)

    if ("nc", le, ld) not in _NC_CACHE:
        _NC_CACHE[("nc", le, ld)] = build(le, ld)
    nc = _NC_CACHE[("nc", le, ld)]
    res = run_bass_kernel_spmd(nc, in_maps, core_ids=list(range(NCORES)),
                               trace=trace)
    out = np.zeros((B, 1), np.float32)
    for c in range(NCORES):
        out[c * BL:(c + 1) * BL, 0] = res.results[c]["y"][0]
    if trace:
        return out, res
    return out


# revision 27
# speedup vs baseline: 1.3212x; 1.0796x over previous
"""Trainium2 Bass kernel for nn_ReachabilityClassifierTransformer.

Data-parallel over batch: 16 samples / 8 cores = 2 samples per core.
Each core runs the full network (6-layer encoder + 4-layer decoder + head)
on its 2 samples. No collectives.

v2 changes over the original baseline:
  - fp16 operands everywhere on the hot path (PE same speed, FWL weight
    loads, 2x DVE rate, half the weight DMA).
  - reciprocal_approx_fast for softmax denominators (~5x faster than
    nc.vector.reciprocal, which was 410us of DVE time on the critical path).
  - LN rstd computed as Exp(-0.5*Ln(var+eps)) so the ACT engine stays on
    the natural_log_exp_and_others table set for the whole kernel (the
    sqrt<->exp set alternation cost ~2.7us per swap, 2 swaps/layer).
  - Sigmoid head computed via Exp + reciprocal (no sigmoid table load).
  - PSUM evacuations rebalanced: biased evacs moved to DVE tensor_scalar
    (one fused op) where ACT was the bottleneck.

Device layout conventions (per core):
  - Activations are kept FEATURE-MAJOR in SBUF: tile [128, KC, T] holds
    X.T, i.e. element [p, k, t] = X[t, k*128+p]. T = 2*512 tokens
    (sample-major concat).
  - All weights are pre-transposed on host to [in_feat, out_feat] and laid
    out as [128, KC_in, O] (partition = in-feature % 128), fp16.
  - matmul(out_psum[M,N], lhsT=[K,M], rhs=[K,N]) computes lhsT.T @ rhs with
    K on partitions.
  - Encoder stage-1 q,k output features are de-interleaved (even feats then
    odd feats) via host-side column permutation of in_proj, so RoPE becomes
    contiguous block ops; the roped result is in natural order again.
  - Softmax: scores are computed transposed (S.T = K_h @ Q_h.T per 128-row
    chunk), exp'd without max subtraction (|scores/8| < 1 for this model),
    and the denominator comes free from a ones-column appended to V.
"""
import numpy as np

import concourse.bass as bass
import concourse.mybir as mybir
import concourse.tile as tile
from concourse import bacc
from concourse.bass_utils import run_bass_kernel_spmd

AF = mybir.ActivationFunctionType
ALU = mybir.AluOpType
F32 = mybir.dt.float32
F32R = mybir.dt.float32r
F16 = mybir.dt.float16

B, S, D, FF, H, LE, LD, M = 16, 512, 512, 2048, 8, 6, 4, 2048
ROPE_BASE = 10000.0
LN_EPS = 1e-5
NCORES = 8
BL = B // NCORES          # 2 samples per core
T = BL * S                # 1024 tokens per core
KC = D // 128             # 4 feature chunks
FC = FF // 128            # 16
MC = M // 128             # 16
DH = D // H               # 64


# ----------------------------------------------------------------------------
# host-side helpers
# ----------------------------------------------------------------------------

def _chunked(wT, dt=np.float16):
    """[Din, O] -> [128, Din//128, O] contiguous."""
    Din, O = wT.shape
    return np.ascontiguousarray(
        wT.reshape(Din // 128, 128, O).transpose(1, 0, 2)).astype(dt)


def _bias_cols(b):
    """[O] -> [128, O//128]  (column per 128-chunk)."""
    O = b.shape[0]
    return np.ascontiguousarray(b.reshape(O // 128, 128).T).astype(np.float32)


_DEINT = np.concatenate([np.arange(0, D, 2), np.arange(1, D, 2)])  # de-interleave


def prep_weights(inp, le=LE, ld=LD):
    """Host-side weight prep -> dict of arrays shared by all cores."""
    out = {}
    g = {k: np.asarray(v, np.float32) for k, v in inp.items()}

    out["mpwT"] = np.ascontiguousarray(g["morph_proj_w"].T)        # [3, 512] f32
    out["mpb"] = _bias_cols(g["morph_proj_b"])                     # [128, 4]
    out["ppwT"] = np.ascontiguousarray(g["pose_proj_w"].T)         # [9, 512] f32
    out["ppb"] = _bias_cols(g["pose_proj_b"])

    # rope grids, de-interleaved frequency order: [128, 2, 512] f16
    freq = 1.0 / ROPE_BASE ** (np.arange(0, D, 2, dtype=np.float64) / D)
    ang = np.outer(np.arange(S, dtype=np.float64), freq)           # [512, 256]
    out["gridc"] = _chunked(np.cos(ang).T.astype(np.float32).reshape(256, S))
    out["grids"] = _chunked(np.sin(ang).T.astype(np.float32).reshape(256, S))

    e_w1, e_w1b, e_w2, e_w2b, e_vb = [], [], [], [], []
    e_ow, e_owb, e_l1, e_l1b, e_l2, e_l2b = [], [], [], [], [], []
    for i in range(le):
        w1 = g["enc_in_w"][i] * g["enc_n1_g"][i][None, :]          # fold n1 g
        b1 = g["enc_in_b"][i] + g["enc_in_w"][i] @ g["enc_n1_b"][i]
        # stage-1: de-interleave q,k output columns
        perm = np.concatenate([_DEINT, D + _DEINT, 2 * D + np.arange(D)])
        e_w1.append(_chunked(np.ascontiguousarray(w1[perm].T)))    # [128,4,1536]
        e_w1b.append(_bias_cols(b1[perm]))                         # [128,12]
        # stage-2 (natural order, raw weights - the faithful quirk)
        w2 = g["enc_in_w"][i][: 2 * D]                             # Wq;Wk
        e_w2.append(_chunked(np.ascontiguousarray(w2.T)))          # [128,4,1024]
        e_w2b.append(_bias_cols(g["enc_in_b"][i][: 2 * D]))        # [128,8]
        e_vb.append(g["enc_in_b"][i][2 * D:][None, :].astype(np.float16))  # [1,512]
        e_ow.append(_chunked(np.ascontiguousarray(g["enc_out_w"][i].T)))
        e_owb.append(_bias_cols(g["enc_out_b"][i]))
        l1 = g["enc_l1_w"][i] * g["enc_n2_g"][i][None, :]
        l1b = g["enc_l1_b"][i] + g["enc_l1_w"][i] @ g["enc_n2_b"][i]
        e_l1.append(_chunked(np.ascontiguousarray(l1.T)))          # [128,4,2048]
        e_l1b.append(_bias_cols(l1b))                              # [128,16]
        e_l2.append(_chunked(np.ascontiguousarray(g["enc_l2_w"][i].T)))
        e_l2b.append(_bias_cols(g["enc_l2_b"][i]))                 # [128,4]
    def _st(lst, shape, dt=np.float16):
        return np.stack(lst) if lst else np.zeros((0,) + shape, dt)
    out["ew1T"], out["ew1b"] = _st(e_w1, (128, KC, 3 * D)), _st(e_w1b, (128, 12), np.float32)
    out["ew2T"], out["ew2b"] = _st(e_w2, (128, KC, 2 * D)), _st(e_w2b, (128, 8), np.float32)
    out["evb"] = _st(e_vb, (1, D))
    out["eowT"], out["eowb"] = _st(e_ow, (128, KC, D)), _st(e_owb, (128, 4), np.float32)
    out["el1T"], out["el1b"] = _st(e_l1, (128, KC, FF)), _st(e_l1b, (128, 16), np.float32)
    out["el2T"], out["el2b"] = _st(e_l2, (128, FC, D)), _st(e_l2b, (128, 4), np.float32)

    d_in, d_inb, d_vb, d_ow, d_owb = [], [], [], [], []
    d_m1, d_m1b, d_m2, d_m2b = [], [], [], []
    for i in range(ld):
        w = g["dec_in_w"][i].copy()
        b = g["dec_in_b"][i].copy()
        w[:D] = w[:D] * g["dec_n1_g"][i][None, :]                  # Wq <- dec_n1
        b[:D] = b[:D] + g["dec_in_w"][i][:D] @ g["dec_n1_b"][i]
        w[D:] = w[D:] * g["enc_final_g"][None, :]                  # Wk,Wv <- enc_final
        b[D:] = b[D:] + g["dec_in_w"][i][D:] @ g["enc_final_b"]
        d_in.append(_chunked(np.ascontiguousarray(w.T)))           # [128,4,1536]
        d_inb.append(_bias_cols(b))
        d_vb.append(b[2 * D:][None, :].astype(np.float16))         # [1,512]
        d_ow.append(_chunked(np.ascontiguousarray(g["dec_out_w"][i].T), np.float32))
        d_owb.append(_bias_cols(g["dec_out_b"][i]))
        m1 = g["dec_m1_w"][i] * g["dec_n2_g"][i][None, :]
        m1b = g["dec_m1_b"][i] + g["dec_m1_w"][i] @ g["dec_n2_b"][i]
        d_m1.append(_chunked(np.ascontiguousarray(m1.T)))          # [128,4,2048]
        d_m1b.append(_bias_cols(m1b))
        d_m2.append(_chunked(np.ascontiguousarray(g["dec_m2_w"][i].T)))
        d_m2b.append(_bias_cols(g["dec_m2_b"][i]))
    out["dinT"], out["dinb"] = _st(d_in, (128, KC, 3 * D)), _st(d_inb, (128, 12), np.float32)
    out["dvb"] = _st(d_vb, (1, D))
    out["dowT"], out["dowb"] = _st(d_ow, (128, KC, D), np.float32), _st(d_owb, (128, 4), np.float32)
    out["dm1T"], out["dm1b"] = _st(d_m1, (128, KC, M)), _st(d_m1b, (128, 16), np.float32)
    out["dm2T"], out["dm2b"] = _st(d_m2, (128, MC, D)), _st(d_m2b, (128, 4), np.float32)

    hw = (g["head_w"] * g["head_g"][None, :])[0]                   # [512]
    hw_pad = np.zeros((128, D // 128, 8), np.float16)
    hw_pad[:, :, 0] = _bias_cols(hw)
    out["hwT"] = hw_pad                                            # [128, 4, 8] f16
    out["hbn"] = -(g["head_bias"] + g["head_w"] @ g["head_b"]).reshape(1, 1)
    return out


# ----------------------------------------------------------------------------
# device program
# ----------------------------------------------------------------------------

def build(le=LE, ld=LD):
    nc = bacc.Bacc(None, target_bir_lowering=False)

    dram = {}

    def din(name, shape, dt=F16):
        dram[name] = nc.dram_tensor(name, list(shape), dt, kind="ExternalInput")
        return dram[name]

    # shared weights
    din("mpwT", [3, 512], F32R); din("mpb", [128, 4], F32)
    din("ppwT", [9, 512], F32R); din("ppb", [128, 4], F32)
    din("gridc", [128, 2, S]); din("grids", [128, 2, S])
    din("ew1T", [le, 128, KC, 3 * D]); din("ew1b", [le, 128, 12], F32)
    din("ew2T", [le, 128, KC, 2 * D]); din("ew2b", [le, 128, 8], F32)
    din("evb", [le, 1, D])
    din("eowT", [le, 128, KC, D]); din("eowb", [le, 128, 4], F32)
    din("el1T", [le, 128, KC, FF]); din("el1b", [le, 128, 16], F32)
    din("el2T", [le, 128, FC, D]); din("el2b", [le, 128, 4], F32)
    din("dinT", [ld, 128, KC, 3 * D]); din("dinb", [ld, 128, 12], F32)
    din("dvb", [ld, 1, D])
    din("dowT", [ld, 128, KC, D], F32R); din("dowb", [ld, 128, 4], F32)
    din("dm1T", [ld, 128, KC, M]); din("dm1b", [ld, 128, 16], F32)
    din("dm2T", [ld, 128, MC, D]); din("dm2b", [ld, 128, 4], F32)
    din("hwT", [128, KC, 8]); din("hbn", [1, 1], F32)
    # per-core inputs
    din("morphT", [3, T], F32R)
    din("poseT", [9, BL], F32R)
    y = nc.dram_tensor("y", [1, BL], F32, kind="ExternalOutput")

    with tile.TileContext(nc) as tc:
        _build_body(nc, tc, dram, y, le, ld)
    nc.compile()
    return nc


def _build_body(nc, tc, dram, y_dram, le, ld):
    import contextlib
    ctx = contextlib.ExitStack()
    with ctx:
        ctx.enter_context(nc.allow_low_precision(
            reason="fp16 operands / approx reciprocal are intentional"))
        persist = ctx.enter_context(tc.tile_pool(name="persist", bufs=1))
        wpool = ctx.enter_context(tc.tile_pool(name="wpool", bufs=3))
        w2pool = ctx.enter_context(tc.tile_pool(name="w2pool", bufs=1))
        owpool = ctx.enter_context(tc.tile_pool(name="owpool", bufs=1))
        bpool = ctx.enter_context(tc.tile_pool(name="bpool", bufs=2))
        a4 = ctx.enter_context(tc.tile_pool(name="a4", bufs=3))
        a8 = ctx.enter_context(tc.tile_pool(name="a8", bufs=2))
        vp = ctx.enter_context(tc.tile_pool(name="vp", bufs=1))
        vpd = ctx.enter_context(tc.tile_pool(name="vpd", bufs=1))
        rtp = ctx.enter_context(tc.tile_pool(name="rtp", bufs=3))
        vbp = ctx.enter_context(tc.tile_pool(name="vbp", bufs=1))
        scr = ctx.enter_context(tc.tile_pool(name="scr", bufs=2))
        scrrc = ctx.enter_context(tc.tile_pool(name="scrrc", bufs=1))
        smalls = ctx.enter_context(tc.tile_pool(name="smalls", bufs=2))
        b1 = ctx.enter_context(tc.tile_pool(name="b1", bufs=4, space="PSUM"))
        b2 = ctx.enter_context(tc.tile_pool(name="b2", bufs=2, space="PSUM"))
        lnp = ctx.enter_context(tc.tile_pool(name="lnp", bufs=1, space="PSUM"))
        sqp = ctx.enter_context(tc.tile_pool(name="sqp", bufs=2))
        qk2p, atp = a8, a4  # share slots/tags

        # ---------------- persistent tiles ----------------
        x = persist.tile([128, KC, T], F16)           # residual stream (X.T)
        vloc_d = persist.tile([128, KC, 8, 65], F32R)  # decoder V (no pool)
        gridc = persist.tile([128, 2, S], F16)
        grids = persist.tile([128, 2, S], F16)
        ones128h = persist.tile([128, 1], F16)
        ones_row = persist.tile([1, 128], F32R)
        ones_rowh = persist.tile([1, 128], F16)
        eps_t = persist.tile([1, 1], F32)
        p = persist.tile([128, KC, BL], F16)          # decoder latent p.T
        nc.sync.dma_start(gridc[:], dram["gridc"][:])
        nc.sync.dma_start(grids[:], dram["grids"][:])
        ones8h = persist.tile([128, 8], F16)
        ones8r = persist.tile([128, 8], F32R)
        stage_f32 = rtp.tile([128, 128], F32, tag="rt")
        nc.vector.memset(stage_f32[:], 1.0)
        nc.vector.tensor_copy(ones128h[:], stage_f32[:, 0:1])
        nc.vector.tensor_copy(ones_row[:], stage_f32[0:1, :])
        nc.vector.tensor_copy(ones_rowh[:], stage_f32[0:1, :])
        nc.vector.tensor_copy(ones8h[:], stage_f32[:, 0:8])
        nc.vector.tensor_copy(ones8r[:], stage_f32[:, 0:8])
        nc.vector.memset(eps_t[:], LN_EPS)

        def c32(ap):
            return ap.bitcast(F32)

        def rstd_from_var(t2):
            """t2 (f32 [1,n]) <- 1/sqrt(t2+eps), via exp(-0.5*ln(t2+eps)).

            Keeps ACT on the natural_log_exp table set (no sqrt set swap)."""
            nc.scalar.activation(t2, t2, AF.Ln, bias=eps_t[:])
            nc.scalar.activation(t2, t2, AF.Exp, scale=-0.5)

        def ln(x_tile, sl, n_tok, h_out, out_sl):
            """h_out[:, :, out_sl] = LayerNorm_features(x_tile[:, :, sl])."""
            sq = a4.tile([128, KC, n_tok], F16, tag="a4")
            for k in range(KC):
                nc.vector.tensor_tensor(sq[:, k, :], x_tile[:, k, sl],
                                        x_tile[:, k, sl], ALU.mult)
            sum_ps = b2.tile([1, n_tok], F32, tag="b2")
            sq_ps = b2.tile([1, n_tok], F32, tag="b2")
            for k in range(KC):
                nc.tensor.matmul(sum_ps[:], ones128h[:], x_tile[:, k, sl],
                                 start=(k == 0), stop=(k == KC - 1))
            for k in range(KC):
                nc.tensor.matmul(sq_ps[:], ones128h[:], sq[:, k, :],
                                 start=(k == 0), stop=(k == KC - 1))
            ms = scr.tile([1, n_tok], F32, tag="scr")
            t2 = scr.tile([1, n_tok], F32, tag="scr")
            rc = scrrc.tile([1, 2, n_tok], F32R, tag="scr_rc")
            nc.scalar.activation(ms[:], sum_ps[:], AF.Copy, scale=1.0 / D)
            nc.vector.tensor_tensor(t2[:], ms[:], ms[:], ALU.mult)      # m^2
            nc.vector.scalar_tensor_tensor(
                t2[:], sq_ps[:], 1.0 / D, t2[:], ALU.mult, ALU.subtract)
            rstd_from_var(t2[:])
            nc.vector.tensor_copy(rc[:, 0, :], t2[:])
            nc.vector.scalar_tensor_tensor(
                rc[:, 1, :], ms[:], -1.0, rc[:, 0, :], ALU.mult, ALU.mult)  # c
            # fp32r misbehaves at tiny moving dims -- bitcast to plain f32
            # for n_tok < 256 (baseline did this via cv()).
            cv = (lambda ap: ap) if n_tok >= 256 else c32
            r_ps = b1.tile([128, n_tok], F32, tag="b1")
            c_ps = b1.tile([128, n_tok], F32, tag="b1")
            nc.tensor.matmul(r_ps[:], cv(ones_row[:]), cv(rc[:, 0, :]),
                             start=True, stop=True)
            nc.tensor.matmul(c_ps[:], cv(ones_row[:]), cv(rc[:, 1, :]),
                             start=True, stop=True)
            for k in range(KC):
                nc.vector.tensor_tensor(h_out[:, k, out_sl], x_tile[:, k, sl],
                                        r_ps[:], ALU.mult)
                nc.vector.tensor_tensor(h_out[:, k, out_sl], h_out[:, k, out_sl],
                                        c_ps[:], ALU.add)

        def ln_stats_pair(x_tile):
            """One LN chain for both samples: returns rc [1, 2, T] f32r."""
            sq = a8.tile([128, KC, T], F16, tag="a8")
            for k in range(KC):
                nc.vector.tensor_tensor(sq[:, k, :], x_tile[:, k, :],
                                        x_tile[:, k, :], ALU.mult)
            sum_ps = b2.tile([1, T], F32, tag="b2")
            sq_ps = b2.tile([1, T], F32, tag="b2")
            for nh in range(BL):
                nsl = slice(nh * S, (nh + 1) * S)
                for k in range(KC):
                    nc.tensor.matmul(sum_ps[:, nsl], ones128h[:],
                                     x_tile[:, k, nsl],
                                     start=(k == 0), stop=(k == KC - 1))
                for k in range(KC):
                    nc.tensor.matmul(sq_ps[:, nsl], ones128h[:], sq[:, k, nsl],
                                     start=(k == 0), stop=(k == KC - 1))
            rc = scrrc.tile([1, 2, T], F32R, tag="scr_rcT")
            ms = rc[:, 1, :]                      # mean parked in the c slot
            t2 = scrrc.tile([1, T], F32, tag="scrT2")
            nc.scalar.activation(ms, sum_ps[:], AF.Copy, scale=1.0 / D)
            nc.vector.tensor_tensor(t2[:], ms, ms, ALU.mult)
            nc.vector.scalar_tensor_tensor(
                t2[:], sq_ps[:], 1.0 / D, t2[:], ALU.mult, ALU.subtract)
            rstd_from_var(t2[:])
            nc.vector.scalar_tensor_tensor(
                rc[:, 1, :], ms, -1.0, t2[:], ALU.mult, ALU.mult)
            nc.vector.tensor_copy(rc[:, 0, :], t2[:])
            return rc

        def ln_apply(rc, x_tile, s, h_out):
            sl = slice(s * S, (s + 1) * S)
            r_ps = b1.tile([128, S], F32, tag="b1")
            c_ps = b1.tile([128, S], F32, tag="b1")
            nc.tensor.matmul(r_ps[:], ones_row[:], rc[:, 0, sl],
                             start=True, stop=True)
            nc.tensor.matmul(c_ps[:], ones_row[:], rc[:, 1, sl],
                             start=True, stop=True)
            for k in range(KC):
                nc.vector.tensor_tensor(h_out[:, k, :], x_tile[:, k, sl],
                                        r_ps[:], ALU.mult)
                nc.vector.tensor_tensor(h_out[:, k, :], h_out[:, k, :],
                                        c_ps[:], ALU.add)

        # ---------------- morph projection -> x ----------------
        morpht = a8.tile([3, T], F32R, tag="a8")
        nc.sync.dma_start(morpht[:], dram["morphT"][:])
        mpw = rtp.tile([3, 512], F32R, tag="rt")
        mpb = rtp.tile([128, 4], F32, tag="rt")
        nc.sync.dma_start(mpw[:], dram["mpwT"][:])
        nc.sync.dma_start(mpb[:], dram["mpb"][:])
        for m in range(KC):
            for s in range(BL):
                ps = b1.tile([128, S], F32, tag="b1")
                nc.tensor.matmul(ps[:], mpw[:, m * 128:(m + 1) * 128],
                                 morpht[:, s * S:(s + 1) * S], start=True, stop=True)
                nc.scalar.activation(x[:, m, s * S:(s + 1) * S], ps[:], AF.Relu,
                                     bias=mpb[:, m:m + 1])

        # ---------------- pose projection -> p ----------------
        poset = rtp.tile([9, BL], F32R, tag="rt")
        ppw = rtp.tile([9, 512], F32R, tag="rt")
        ppb = rtp.tile([128, 4], F32, tag="rt")
        nc.sync.dma_start(poset[:], dram["poseT"][:])
        nc.sync.dma_start(ppw[:], dram["ppwT"][:])
        nc.sync.dma_start(ppb[:], dram["ppb"][:])
        pps = b1.tile([128, KC, BL], F32, tag="b1")
        for m in range(KC):
            nc.tensor.matmul(pps[:, m, :], c32(ppw[:, m * 128:(m + 1) * 128]),
                             c32(poset[:]), start=True, stop=True)
        for m in range(KC):
            nc.scalar.activation(p[:, m, :], pps[:, m, :], AF.Relu,
                                 bias=ppb[:, m:m + 1])

        # ------------- encoder layers (sample-pipelined LN) -------------
        # Per-sample LN stats are issued inside the previous block's
        # instruction stream (deferred via `pend`), so the small stats
        # chain overlaps another sample's PE work and the PE never idles
        # across a LayerNorm boundary (keeps the HAM clock warm).
        pend = []

        def flush_pend():
            while pend:
                pend.pop(0)()

        def ln_sums(s):
            sl = slice(s * S, (s + 1) * S)
            sq = sqp.tile([128, KC, S], F16, tag="sq")
            for k in range(KC):
                nc.vector.tensor_tensor(sq[:, k, :], x[:, k, sl],
                                        x[:, k, sl], ALU.mult)
            sum_ps = lnp.tile([1, S], F32, tag="lnsum")
            sq_ps = lnp.tile([1, S], F32, tag="lnsq")
            for k in range(KC):
                nc.tensor.matmul(sum_ps[:], ones128h[:], x[:, k, sl],
                                 start=(k == 0), stop=(k == KC - 1))
            for k in range(KC):
                nc.tensor.matmul(sq_ps[:], ones128h[:], sq[:, k, :],
                                 start=(k == 0), stop=(k == KC - 1))
            return sum_ps, sq_ps

        def ln_finish(sums, tag):
            sum_ps, sq_ps = sums
            rc = scrrc.tile([1, 2, S], F32R, tag=tag)
            ms = rc[:, 1, :]
            t2 = scr.tile([1, S], F32, tag="lnt2")
            nc.scalar.activation(ms, sum_ps[:], AF.Copy, scale=1.0 / D)
            nc.vector.tensor_tensor(t2[:], ms, ms, ALU.mult)
            nc.vector.scalar_tensor_tensor(
                t2[:], sq_ps[:], 1.0 / D, t2[:], ALU.mult, ALU.subtract)
            rstd_from_var(t2[:])
            nc.vector.scalar_tensor_tensor(
                rc[:, 1, :], ms, -1.0, t2[:], ALU.mult, ALU.mult)
            nc.vector.tensor_copy(rc[:, 0, :], t2[:])
            return rc

        def ln_stats_one(s, tag):
            return ln_finish(ln_sums(s), tag)

        def ln_apply_one(rc, s, h_out):
            sl = slice(s * S, (s + 1) * S)
            r_ps = b1.tile([128, S], F32, tag="b1")
            c_ps = b1.tile([128, S], F32, tag="b1")
            nc.tensor.matmul(r_ps[:], ones_row[:], rc[:, 0, :],
                             start=True, stop=True)
            nc.tensor.matmul(c_ps[:], ones_row[:], rc[:, 1, :],
                             start=True, stop=True)
            for k in range(KC):
                nc.vector.tensor_tensor(h_out[:, k, :], x[:, k, sl],
                                        r_ps[:], ALU.mult)
                nc.vector.tensor_tensor(h_out[:, k, :], h_out[:, k, :],
                                        c_ps[:], ALU.add)

        rc_a = [ln_stats_one(s, f"rcA{s}") for s in range(BL)]
        rc_f = [None, None]

        for li in range(le):
            w1 = wpool.tile([128, KC, 3 * D], F16, tag="bigw")
            nc.sync.dma_start(w1[:], dram["ew1T"][li])
            w1b = bpool.tile([128, 12], F32, tag="w1b")
            nc.sync.dma_start(w1b[:], dram["ew1b"][li])
            w2 = w2pool.tile([128, KC, 2 * D], F16, tag="w2")
            nc.sync.dma_start(w2[:], dram["ew2T"][li])
            w2b = bpool.tile([128, 8], F32, tag="w2b")
            nc.sync.dma_start(w2b[:], dram["ew2b"][li])
            vbrow = bpool.tile([1, D], F16, tag="vbrow")
            nc.sync.dma_start(vbrow[:], dram["evb"][li])
            ow = owpool.tile([128, KC, D], F16, tag="ow")
            nc.sync.dma_start(ow[:], dram["eowT"][li])
            owb = bpool.tile([128, 4], F32, tag="owb")
            nc.sync.dma_start(owb[:], dram["eowb"][li])
            l1 = wpool.tile([128, KC, FF], F16, tag="bigw")
            nc.sync.dma_start(l1[:], dram["el1T"][li])
            l1b = bpool.tile([128, 16], F32, tag="l1b")
            nc.sync.dma_start(l1b[:], dram["el1b"][li])
            l2b = bpool.tile([128, 4], F32, tag="l2b")
            nc.sync.dma_start(l2b[:], dram["el2b"][li])

            # v-bias broadcast [128, 512] (token-major V bias), once per layer
            vb_ps = b1.tile([128, D], F32, tag="b1")
            nc.tensor.matmul(vb_ps[:], ones_rowh[:], vbrow[:], start=True, stop=True)
            vb_bc = vbp.tile([128, D], F32, tag="vb_bc")
            nc.scalar.activation(vb_bc[:], vb_ps[:], AF.Copy)

            for s in range(BL):
                sl = slice(s * S, (s + 1) * S)
                h = a4.tile([128, KC, S], F16, tag="a4")
                ln_apply_one(rc_a[s], s, h)
                # ---- stage 1: q,k (permuted) + v ----
                qkv1 = a8.tile([128, 8, S], F16, tag="a8")
                v1 = a4.tile([128, KC, S], F16, tag="a4")
                for m in range(12):
                    ps = b1.tile([128, S], F32, tag="b1")
                    for k in range(KC):
                        nc.tensor.matmul(ps[:], w1[:, k, m * 128:(m + 1) * 128],
                                         h[:, k, :], start=(k == 0), stop=(k == KC - 1))
                    dest = qkv1[:, m, :] if m < 8 else v1[:, m - 8, :]
                    nc.vector.tensor_scalar_add(dest, ps[:], w1b[:, m:m + 1])
                flush_pend()
                # ---- rope: qkv1 (chunks 0-7) -> qkr (natural order) ----
                qkr = a8.tile([128, 8, S], F16, tag="a8")
                for half in (0, 4):
                    for c in range(2):
                        e = qkv1[:, half + c, :]
                        o = qkv1[:, half + 2 + c, :]
                        r1 = qkr[:, half + c, :]
                        r2 = qkr[:, half + 2 + c, :]
                        t1 = rtp.tile([128, S], F16, tag="rt")
                        nc.vector.tensor_tensor(r1, e, gridc[:, c, :], ALU.mult)
                        nc.vector.tensor_tensor(t1[:], o, grids[:, c, :], ALU.mult)
                        nc.vector.tensor_tensor(r1, r1, t1[:], ALU.subtract)
                        t2 = rtp.tile([128, S], F16, tag="rt")
                        nc.vector.tensor_tensor(r2, e, grids[:, c, :], ALU.mult)
                        nc.vector.tensor_tensor(t2[:], o, gridc[:, c, :], ALU.mult)
                        nc.vector.tensor_tensor(r2, r2, t2[:], ALU.add)
                # ---- stage 2: Q,K ----
                qk2 = qk2p.tile([128, 8, S], F16, tag="a8")
                for m in range(8):
                    ps = b1.tile([128, S], F32, tag="b1")
                    base = 0 if m < 4 else 4
                    for k in range(KC):
                        nc.tensor.matmul(ps[:], w2[:, k, m * 128:(m + 1) * 128],
                                         qkr[:, base + k, :],
                                         start=(k == 0), stop=(k == KC - 1))
                    nc.scalar.activation(qk2[:, m, :], ps[:], AF.Identity,
                                         bias=w2b[:, m:m + 1])
                # ---- stage 2: V (token-major, with ones column per head) ----
                # last dim padded 65->72 so per-head slices stay 16B-aligned
                # in fp16 (65*2B = 130B stride broke odd-head matmul reads)
                vloc = vp.tile([128, KC, 8, 72], F16, tag="vloc")
                for t in range(KC):
                    nc.vector.tensor_copy(vloc[:, t, :, 64], ones8h[:])
                for t in range(KC):
                    ps = b1.tile([128, S], F32, tag="b1")
                    for k in range(KC):
                        nc.tensor.matmul(
                            ps[:], v1[:, k, t * 128:(t + 1) * 128],
                            w1[:, k, 2 * D:3 * D],
                            start=(k == 0), stop=(k == KC - 1))
                    nc.vector.tensor_tensor(
                        vloc[:, t, :, 0:64],
                        ps[:].rearrange("p (h d) -> p h d", h=H),
                        vb_bc[:].rearrange("p (h d) -> p h d", h=H), ALU.add)
                # ---- attention heads (paired: exp(h+1) hides under A@V(h)) ----
                o_t = a4.tile([128, KC, S], F16, tag="a4")
                for h0 in range(0, H, 2):
                    ats = {}
                    for hh in (h0, h0 + 1):
                        rows = slice(64 * (hh % 2), 64 * (hh % 2) + 64)
                        at = atp.tile([128, KC, S], F16, tag="a4")
                        for c in range(KC):
                            scp = b1.tile([128, S], F32, tag="b1")
                            nc.tensor.matmul(
                                scp[:],
                                qk2[rows, 4 + hh // 2, c * 128:(c + 1) * 128],
                                qk2[rows, hh // 2, :], start=True, stop=True)
                            nc.scalar.activation(at[:, c, :], scp[:], AF.Exp,
                                                 scale=float(1.0 / np.sqrt(DH)))
                        ats[hh] = at
                    for hh in (h0, h0 + 1):
                        rows = slice(64 * (hh % 2), 64 * (hh % 2) + 64)
                        at = ats[hh]
                        ov = b2.tile([65, S], F32, tag="b2")
                        for c in range(KC):
                            nc.tensor.matmul(ov[:], vloc[:, c, hh, 0:65],
                                             at[:, c, :],
                                             start=(c == 0), stop=(c == KC - 1))
                        # custom DVE recip mishandles nonzero psum base
                        # partition: stage the denominator row to SBUF p0
                        # via ACT Copy (regular op) first.
                        t_den = scr.tile([1, S], F32, tag="scr")
                        nc.scalar.activation(t_den[:], ov[64:65, :], AF.Copy)
                        rec = scr.tile([1, S], F32, tag="scr")
                        nc.vector.reciprocal_approx_fast(rec[:], t_den[:])
                        rec_h = scr.tile([1, S], F16, tag="scr")
                        nc.vector.tensor_copy(rec_h[:], rec[:])
                        rb = b2.tile([64, S], F32, tag="b2")
                        nc.tensor.matmul(rb[:], ones_rowh[:, 0:64], rec_h[:],
                                         start=True, stop=True)
                        rb_sb = scr.tile([64, S], F16, tag="scr")
                        nc.scalar.activation(rb_sb[:], rb[:], AF.Copy)
                        nc.vector.tensor_tensor(o_t[rows, hh // 2, :],
                                                ov[0:64, :], rb_sb[:], ALU.mult)
                # ---- out-proj + residual ----
                for m in range(KC):
                    ps = b1.tile([128, S], F32, tag="b1")
                    for k in range(KC):
                        nc.tensor.matmul(ps[:], ow[:, k, m * 128:(m + 1) * 128],
                                         o_t[:, k, :], start=(k == 0),
                                         stop=(k == KC - 1))
                    nc.vector.scalar_tensor_tensor(
                        x[:, m, sl], ps[:], owb[:, m:m + 1], x[:, m, sl],
                        ALU.add, ALU.add)

                def _mkf(s=s):
                    rc_f[s] = ln_stats_one(s, f"rcF{s}")
                pend.append(_mkf)
            # ---- phase B: l2 streams in (chunked) once w1 is released ----
            l2 = wpool.tile([128, FC, D], F16, tag="bigw")
            for kf in range(FC):
                nc.sync.dma_start(l2[:, kf, :], dram["el2T"][li][:, kf, :])
            for s in range(BL):
                sl = slice(s * S, (s + 1) * S)
                h2 = a4.tile([128, KC, S], F16, tag="a4")
                ln_apply_one(rc_f[s], s, h2)
                # ---- FFN ----
                f2 = [b1.tile([128, S], F32, tag="b1", name=f"f2_{_m}")
                      for _m in range(KC)]
                for kf in range(FC):
                    if kf == 4:
                        flush_pend()
                    f1 = b2.tile([128, S], F32, tag="b2")
                    for k in range(KC):
                        nc.tensor.matmul(f1[:], l1[:, k, kf * 128:(kf + 1) * 128],
                                         h2[:, k, :], start=(k == 0),
                                         stop=(k == KC - 1))
                    rt = rtp.tile([128, S], F16, tag="rt")
                    nc.vector.tensor_scalar(rt[:], f1[:], l1b[:, kf:kf + 1], 0.0,
                                            ALU.add, ALU.max)
                    for m in range(KC):
                        nc.tensor.matmul(f2[m][:], l2[:, kf, m * 128:(m + 1) * 128],
                                         rt[:], start=(kf == 0), stop=(kf == FC - 1))
                for m in range(KC):
                    nc.vector.scalar_tensor_tensor(
                        x[:, m, sl], f2[m][:], l2b[:, m:m + 1], x[:, m, sl],
                        ALU.add, ALU.add)

                def _mka(s=s):
                    rc_a[s] = ln_stats_one(s, f"rcA{s}")
                pend.append(_mka)

        # ---------------- final encoder LN (in-place; affine folded) --------
        me = x
        flush_pend()
        for s in range(BL):
            ln_apply_one(rc_a[s], s, x[:, :, s * S:(s + 1) * S])

        # ---------------- decoder layers ----------------
        for li in range(ld):
            dw = wpool.tile([128, KC, 3 * D], F16, tag="bigw")
            nc.sync.dma_start(dw[:], dram["dinT"][li])
            dwb = bpool.tile([128, 12], F32, tag="w1b")
            nc.sync.dma_start(dwb[:], dram["dinb"][li])
            dvbrow = bpool.tile([1, D], F16, tag="vbrow")
            nc.sync.dma_start(dvbrow[:], dram["dvb"][li])
            do = owpool.tile([128, KC, D], F32R, tag="ow")
            nc.sync.dma_start(do[:], dram["dowT"][li])
            dob = bpool.tile([128, 4], F32, tag="owb")
            nc.sync.dma_start(dob[:], dram["dowb"][li])
            m1 = wpool.tile([128, KC, M], F16, tag="bigw")
            nc.sync.dma_start(m1[:], dram["dm1T"][li])
            m1b = bpool.tile([128, 16], F32, tag="l1b")
            nc.sync.dma_start(m1b[:], dram["dm1b"][li])
            m2b = bpool.tile([128, 4], F32, tag="l2b")
            nc.sync.dma_start(m2b[:], dram["dm2b"][li])

            vb_ps = b1.tile([128, D], F32, tag="b1")
            nc.tensor.matmul(vb_ps[:], ones_rowh[:], dvbrow[:], start=True, stop=True)
            vb_bc = vbp.tile([128, D], F32, tag="vb_bc")
            nc.scalar.activation(vb_bc[:], vb_ps[:], AF.Copy)

            # LN(p) -> q_ln ; Q projection (all samples at once, N=BL)
            q_ln = smalls.tile([128, KC, 8], F16, tag="q_ln")
            ln(p, slice(None), BL, q_ln, slice(0, BL))
            qps = b1.tile([128, KC, BL], F32, tag="b1")
            for m in range(KC):
                for k in range(KC):
                    nc.tensor.matmul(qps[:, m, :],
                                     dw[:, k, m * 128:(m + 1) * 128],
                                     q_ln[:, k, 0:BL], start=(k == 0),
                                     stop=(k == KC - 1))
            q_sb = smalls.tile([128, KC, BL], F32R, tag="q_sb")
            for m in range(KC):
                nc.scalar.activation(q_sb[:, m, :], qps[:, m, :], AF.Identity,
                                     bias=dwb[:, m:m + 1])
            o_d = smalls.tile([128, KC, BL], F32R, tag="o_d")
            for s in range(BL):
                sl = slice(s * S, (s + 1) * S)
                # K (feature-major) and V' (token-major) over morph_enc
                k_sb = a4.tile([128, KC, S], F32R, tag="a4")
                for m in range(KC):
                    ps = b1.tile([128, S], F32, tag="b1")
                    for k in range(KC):
                        nc.tensor.matmul(
                            ps[:], dw[:, k, D + m * 128:D + (m + 1) * 128],
                            me[:, k, sl], start=(k == 0), stop=(k == KC - 1))
                    nc.scalar.activation(k_sb[:, m, :], ps[:], AF.Identity,
                                         bias=dwb[:, 4 + m:5 + m])
                vloc = vloc_d
                for t in range(KC):
                    nc.vector.tensor_copy(vloc[:, t, :, 64], ones8r[:])
                for t in range(KC):
                    ps = b1.tile([128, S], F32, tag="b1")
                    for k in range(KC):
                        nc.tensor.matmul(
                            ps[:], me[:, k, s * S + t * 128:s * S + (t + 1) * 128],
                            dw[:, k, 2 * D:3 * D],
                            start=(k == 0), stop=(k == KC - 1))
                    nc.vector.tensor_tensor(
                        vloc[:, t, :, 0:64],
                        ps[:].rearrange("p (h d) -> p h d", h=H),
                        vb_bc[:].rearrange("p (h d) -> p h d", h=H), ALU.add)
                scp = b1.tile([128, KC, H], F32, tag="b1")
                for hh in range(H):
                    rows = slice(64 * (hh % 2), 64 * (hh % 2) + 64)
                    for c in range(KC):
                        nc.tensor.matmul(
                            scp[:, c, hh:hh + 1],
                            c32(k_sb[rows, hh // 2, c * 128:(c + 1) * 128]),
                            c32(q_sb[rows, hh // 2, s:s + 1]),
                            start=True, stop=True)
                at = smalls.tile([128, KC, H], F32R, tag="at_d")
                nc.scalar.activation(at[:], scp[:], AF.Exp,
                                     scale=float(1.0 / np.sqrt(DH)))
                ov = b2.tile([65, H], F32, tag="b2")
                for hh in range(H):
                    for c in range(KC):
                        nc.tensor.matmul(ov[:, hh:hh + 1], c32(vloc[:, c, hh, :]),
                                         c32(at[:, c, hh:hh + 1]),
                                         start=(c == 0), stop=(c == KC - 1))
                rec = scr.tile([1, H], F32R, tag="scr")
                nc.vector.reciprocal(rec[:], ov[64:65, :])
                rb = b2.tile([64, H], F32, tag="b2")
                nc.tensor.matmul(rb[:], c32(ones_row[:, 0:64]), c32(rec[:]),
                                 start=True, stop=True)
                rb_sb = scr.tile([64, H], F32, tag="scr")
                nc.scalar.activation(rb_sb[:], rb[:], AF.Copy)
                for hh in range(H):
                    rows = slice(64 * (hh % 2), 64 * (hh % 2) + 64)
                    nc.vector.tensor_tensor(o_d[rows, hh // 2, s:s + 1],
                                            ov[0:64, hh:hh + 1],
                                            rb_sb[:, hh:hh + 1], ALU.mult)
            # out-proj + residual into p
            ops = b1.tile([128, KC, BL], F32, tag="b1")
            for m in range(KC):
                for k in range(KC):
                    nc.tensor.matmul(ops[:, m, :],
                                     c32(do[:, k, m * 128:(m + 1) * 128]),
                                     c32(o_d[:, k, :]), start=(k == 0),
                                     stop=(k == KC - 1))
            for m in range(KC):
                nc.vector.scalar_tensor_tensor(
                    p[:, m, :], ops[:, m, :], dob[:, m:m + 1], p[:, m, :],
                    ALU.add, ALU.add)
            # FFN on p (m2 streams in chunked once dw releases its slot)
            m2 = wpool.tile([128, MC, D], F16, tag="bigw")
            for kf in range(MC):
                nc.sync.dma_start(m2[:, kf, :], dram["dm2T"][li][:, kf, :])
            h2d = smalls.tile([128, KC, 8], F16, tag="q_ln")
            ln(p, slice(None), BL, h2d, slice(0, BL))
            mh = smalls.tile([128, MC, 8], F16, tag="mh")
            for mm_ in range(MC):
                ps = b1.tile([128, BL], F32, tag="b1")
                for k in range(KC):
                    nc.tensor.matmul(ps[:], m1[:, k, mm_ * 128:(mm_ + 1) * 128],
                                     h2d[:, k, 0:BL], start=(k == 0),
                                     stop=(k == KC - 1))
                nc.vector.tensor_scalar(mh[:, mm_, 0:BL], ps[:],
                                        m1b[:, mm_:mm_ + 1],
                                        0.0, ALU.add, ALU.max)
            m2ps = b1.tile([128, KC, BL], F32, tag="b1")
            for m in range(KC):
                for kf in range(MC):
                    nc.tensor.matmul(m2ps[:, m, :],
                                     m2[:, kf, m * 128:(m + 1) * 128],
                                     mh[:, kf, 0:BL], start=(kf == 0),
                                     stop=(kf == MC - 1))
            for m in range(KC):
                nc.vector.scalar_tensor_tensor(
                    p[:, m, :], m2ps[:, m, :], m2b[:, m:m + 1], p[:, m, :],
                    ALU.add, ALU.add)

        # ---------------- head (sigmoid via exp + recip; no table swap) -----
        hw = smalls.tile([128, KC, 8], F16, tag="hw")
        hbn = smalls.tile([1, 1], F32, tag="hb")
        nc.sync.dma_start(hw[:], dram["hwT"][:])
        nc.sync.dma_start(hbn[:], dram["hbn"][:])
        hg = smalls.tile([128, KC, 8], F16, tag="q_ln")
        ln(p, slice(None), BL, hg, slice(0, BL))
        hps = b2.tile([1, BL], F32, tag="b2")
        for k in range(KC):
            nc.tensor.matmul(hps[:], hw[:, k, 0:1], hg[:, k, 0:BL],
                             start=(k == 0), stop=(k == KC - 1))
        e_t = smalls.tile([1, BL], F32, tag="y_e")
        nc.scalar.activation(e_t[:], hps[:], AF.Exp, scale=-1.0, bias=hbn[:])
        nc.vector.tensor_scalar_add(e_t[:], e_t[:], 1.0)
        y_sb = smalls.tile([1, BL], F32, tag="y_sb")
        nc.vector.reciprocal_approx_fast(y_sb[:], e_t[:])
        nc.sync.dma_start(y_dram[:], y_sb[:])


# ----------------------------------------------------------------------------
# entry point
# ----------------------------------------------------------------------------

_NC_CACHE = {}


def kernel(**inputs):
    return _run(inputs, LE, LD)


def _run(inputs, le, ld, trace=False):
    w = prep_weights(inputs, le, ld)
    morph = np.asarray(inputs["morph"], np.float32)
    pose = np.asarray(inputs["pose"], np.float32)
    in_maps = []
    for c in range(NCORES):
        im = dict(w)
        mo = morph[c * BL:(c + 1) * BL]                 # [BL, S, 3]
        im["morphT"] = np.ascontiguousarray(
            mo.transpose(2, 0, 1).reshape(3, T))
        im["poseT"] = np.ascontiguousarray(pose[c * BL:(c + 1) * BL].T)
        in_maps.append(im)

    if ("nc", le, ld) not in _NC_CACHE:
        _NC_CACHE[("nc", le, ld)] = build(le, ld)
    nc = _NC_CACHE[("nc", le, ld)]
    res = run_bass_kernel_spmd(nc, in_maps, core_ids=list(range(NCORES)),
                               trace=trace)
    out = np.zeros((B, 1), np.float32)
    for c in range(NCORES):
        out[c * BL:(c + 1) * BL, 0] = res.results[c]["y"][0]
    if trace:
        return out, res
    return out


# revision 29
# speedup vs baseline: 1.3243x; 1.0024x over previous
"""Trainium2 Bass kernel for nn_ReachabilityClassifierTransformer.

Data-parallel over batch: 16 samples / 8 cores = 2 samples per core.
Each core runs the full network (6-layer encoder + 4-layer decoder + head)
on its 2 samples. No collectives.

v2 changes over the original baseline:
  - fp16 operands everywhere on the hot path (PE same speed, FWL weight
    loads, 2x DVE rate, half the weight DMA).
  - reciprocal_approx_fast for softmax denominators (~5x faster than
    nc.vector.reciprocal, which was 410us of DVE time on the critical path).
  - LN rstd computed as Exp(-0.5*Ln(var+eps)) so the ACT engine stays on
    the natural_log_exp_and_others table set for the whole kernel (the
    sqrt<->exp set alternation cost ~2.7us per swap, 2 swaps/layer).
  - Sigmoid head computed via Exp + reciprocal (no sigmoid table load).
  - PSUM evacuations rebalanced: biased evacs moved to DVE tensor_scalar
    (one fused op) where ACT was the bottleneck.

Device layout conventions (per core):
  - Activations are kept FEATURE-MAJOR in SBUF: tile [128, KC, T] holds
    X.T, i.e. element [p, k, t] = X[t, k*128+p]. T = 2*512 tokens
    (sample-major concat).
  - All weights are pre-transposed on host to [in_feat, out_feat] and laid
    out as [128, KC_in, O] (partition = in-feature % 128), fp16.
  - matmul(out_psum[M,N], lhsT=[K,M], rhs=[K,N]) computes lhsT.T @ rhs with
    K on partitions.
  - Encoder stage-1 q,k output features are de-interleaved (even feats then
    odd feats) via host-side column permutation of in_proj, so RoPE becomes
    contiguous block ops; the roped result is in natural order again.
  - Softmax: scores are computed transposed (S.T = K_h @ Q_h.T per 128-row
    chunk), exp'd without max subtraction (|scores/8| < 1 for this model),
    and the denominator comes free from a ones-column appended to V.
"""
import numpy as np

import concourse.bass as bass
import concourse.mybir as mybir
import concourse.tile as tile
from concourse import bacc
from concourse.bass_utils import run_bass_kernel_spmd

AF = mybir.ActivationFunctionType
ALU = mybir.AluOpType
F32 = mybir.dt.float32
F32R = mybir.dt.float32r
F16 = mybir.dt.float16

B, S, D, FF, H, LE, LD, M = 16, 512, 512, 2048, 8, 6, 4, 2048
ROPE_BASE = 10000.0
LN_EPS = 1e-5
NCORES = 8
BL = B // NCORES          # 2 samples per core
T = BL * S                # 1024 tokens per core
KC = D // 128             # 4 feature chunks
FC = FF // 128            # 16
MC = M // 128             # 16
DH = D // H               # 64


# ----------------------------------------------------------------------------
# host-side helpers
# ----------------------------------------------------------------------------

def _chunked(wT, dt=np.float16):
    """[Din, O] -> [128, Din//128, O] contiguous."""
    Din, O = wT.shape
    return np.ascontiguousarray(
        wT.reshape(Din // 128, 128, O).transpose(1, 0, 2)).astype(dt)


def _bias_cols(b):
    """[O] -> [128, O//128]  (column per 128-chunk)."""
    O = b.shape[0]
    return np.ascontiguousarray(b.reshape(O // 128, 128).T).astype(np.float32)


_DEINT = np.concatenate([np.arange(0, D, 2), np.arange(1, D, 2)])  # de-interleave


def prep_weights(inp, le=LE, ld=LD):
    """Host-side weight prep -> dict of arrays shared by all cores."""
    out = {}
    g = {k: np.asarray(v, np.float32) for k, v in inp.items()}

    out["mpwT"] = np.ascontiguousarray(g["morph_proj_w"].T)        # [3, 512] f32
    out["mpb"] = _bias_cols(g["morph_proj_b"])                     # [128, 4]
    out["ppwT"] = np.ascontiguousarray(g["pose_proj_w"].T)         # [9, 512] f32
    out["ppb"] = _bias_cols(g["pose_proj_b"])

    # rope grids, de-interleaved frequency order: [128, 2, 512] f16
    freq = 1.0 / ROPE_BASE ** (np.arange(0, D, 2, dtype=np.float64) / D)
    ang = np.outer(np.arange(S, dtype=np.float64), freq)           # [512, 256]
    out["gridc"] = _chunked(np.cos(ang).T.astype(np.float32).reshape(256, S))
    out["grids"] = _chunked(np.sin(ang).T.astype(np.float32).reshape(256, S))

    e_w1, e_w1b, e_w2, e_w2b, e_vb = [], [], [], [], []
    e_ow, e_owb, e_l1, e_l1b, e_l2, e_l2b = [], [], [], [], [], []
    for i in range(le):
        w1 = g["enc_in_w"][i] * g["enc_n1_g"][i][None, :]          # fold n1 g
        b1 = g["enc_in_b"][i] + g["enc_in_w"][i] @ g["enc_n1_b"][i]
        # stage-1: de-interleave q,k output columns
        perm = np.concatenate([_DEINT, D + _DEINT, 2 * D + np.arange(D)])
        e_w1.append(_chunked(np.ascontiguousarray(w1[perm].T)))    # [128,4,1536]
        e_w1b.append(_bias_cols(b1[perm]))                         # [128,12]
        # stage-2 (natural order, raw weights - the faithful quirk)
        w2 = g["enc_in_w"][i][: 2 * D]                             # Wq;Wk
        e_w2.append(_chunked(np.ascontiguousarray(w2.T)))          # [128,4,1024]
        e_w2b.append(_bias_cols(g["enc_in_b"][i][: 2 * D]))        # [128,8]
        e_vb.append(g["enc_in_b"][i][2 * D:][None, :].astype(np.float16))  # [1,512]
        e_ow.append(_chunked(np.ascontiguousarray(g["enc_out_w"][i].T)))
        e_owb.append(_bias_cols(g["enc_out_b"][i]))
        l1 = g["enc_l1_w"][i] * g["enc_n2_g"][i][None, :]
        l1b = g["enc_l1_b"][i] + g["enc_l1_w"][i] @ g["enc_n2_b"][i]
        e_l1.append(_chunked(np.ascontiguousarray(l1.T)))          # [128,4,2048]
        e_l1b.append(_bias_cols(l1b))                              # [128,16]
        e_l2.append(_chunked(np.ascontiguousarray(g["enc_l2_w"][i].T)))
        e_l2b.append(_bias_cols(g["enc_l2_b"][i]))                 # [128,4]
    def _st(lst, shape, dt=np.float16):
        return np.stack(lst) if lst else np.zeros((0,) + shape, dt)
    out["ew1T"], out["ew1b"] = _st(e_w1, (128, KC, 3 * D)), _st(e_w1b, (128, 12), np.float32)
    out["ew2T"], out["ew2b"] = _st(e_w2, (128, KC, 2 * D)), _st(e_w2b, (128, 8), np.float32)
    out["evb"] = _st(e_vb, (1, D))
    out["eowT"], out["eowb"] = _st(e_ow, (128, KC, D)), _st(e_owb, (128, 4), np.float32)
    out["el1T"], out["el1b"] = _st(e_l1, (128, KC, FF)), _st(e_l1b, (128, 16), np.float32)
    out["el2T"], out["el2b"] = _st(e_l2, (128, FC, D)), _st(e_l2b, (128, 4), np.float32)

    d_in, d_inb, d_vb, d_ow, d_owb = [], [], [], [], []
    d_m1, d_m1b, d_m2, d_m2b = [], [], [], []
    for i in range(ld):
        w = g["dec_in_w"][i].copy()
        b = g["dec_in_b"][i].copy()
        w[:D] = w[:D] * g["dec_n1_g"][i][None, :]                  # Wq <- dec_n1
        b[:D] = b[:D] + g["dec_in_w"][i][:D] @ g["dec_n1_b"][i]
        w[D:] = w[D:] * g["enc_final_g"][None, :]                  # Wk,Wv <- enc_final
        b[D:] = b[D:] + g["dec_in_w"][i][D:] @ g["enc_final_b"]
        d_in.append(_chunked(np.ascontiguousarray(w.T)))           # [128,4,1536]
        d_inb.append(_bias_cols(b))
        d_vb.append(b[2 * D:][None, :].astype(np.float16))         # [1,512]
        d_ow.append(_chunked(np.ascontiguousarray(g["dec_out_w"][i].T), np.float32))
        d_owb.append(_bias_cols(g["dec_out_b"][i]))
        m1 = g["dec_m1_w"][i] * g["dec_n2_g"][i][None, :]
        m1b = g["dec_m1_b"][i] + g["dec_m1_w"][i] @ g["dec_n2_b"][i]
        d_m1.append(_chunked(np.ascontiguousarray(m1.T)))          # [128,4,2048]
        d_m1b.append(_bias_cols(m1b))
        d_m2.append(_chunked(np.ascontiguousarray(g["dec_m2_w"][i].T)))
        d_m2b.append(_bias_cols(g["dec_m2_b"][i]))
    out["dinT"], out["dinb"] = _st(d_in, (128, KC, 3 * D)), _st(d_inb, (128, 12), np.float32)
    out["dvb"] = _st(d_vb, (1, D))
    out["dowT"], out["dowb"] = _st(d_ow, (128, KC, D), np.float32), _st(d_owb, (128, 4), np.float32)
    out["dm1T"], out["dm1b"] = _st(d_m1, (128, KC, M)), _st(d_m1b, (128, 16), np.float32)
    out["dm2T"], out["dm2b"] = _st(d_m2, (128, MC, D)), _st(d_m2b, (128, 4), np.float32)

    hw = (g["head_w"] * g["head_g"][None, :])[0]                   # [512]
    hw_pad = np.zeros((128, D // 128, 8), np.float16)
    hw_pad[:, :, 0] = _bias_cols(hw)
    out["hwT"] = hw_pad                                            # [128, 4, 8] f16
    out["hbn"] = -(g["head_bias"] + g["head_w"] @ g["head_b"]).reshape(1, 1)
    return out


# ----------------------------------------------------------------------------
# device program
# ----------------------------------------------------------------------------

def build(le=LE, ld=LD):
    nc = bacc.Bacc(None, target_bir_lowering=False)

    dram = {}

    def din(name, shape, dt=F16):
        dram[name] = nc.dram_tensor(name, list(shape), dt, kind="ExternalInput")
        return dram[name]

    # shared weights
    din("mpwT", [3, 512], F32R); din("mpb", [128, 4], F32)
    din("ppwT", [9, 512], F32R); din("ppb", [128, 4], F32)
    din("gridc", [128, 2, S]); din("grids", [128, 2, S])
    din("ew1T", [le, 128, KC, 3 * D]); din("ew1b", [le, 128, 12], F32)
    din("ew2T", [le, 128, KC, 2 * D]); din("ew2b", [le, 128, 8], F32)
    din("evb", [le, 1, D])
    din("eowT", [le, 128, KC, D]); din("eowb", [le, 128, 4], F32)
    din("el1T", [le, 128, KC, FF]); din("el1b", [le, 128, 16], F32)
    din("el2T", [le, 128, FC, D]); din("el2b", [le, 128, 4], F32)
    din("dinT", [ld, 128, KC, 3 * D]); din("dinb", [ld, 128, 12], F32)
    din("dvb", [ld, 1, D])
    din("dowT", [ld, 128, KC, D], F32R); din("dowb", [ld, 128, 4], F32)
    din("dm1T", [ld, 128, KC, M]); din("dm1b", [ld, 128, 16], F32)
    din("dm2T", [ld, 128, MC, D]); din("dm2b", [ld, 128, 4], F32)
    din("hwT", [128, KC, 8]); din("hbn", [1, 1], F32)
    # per-core inputs
    din("morphT", [3, T], F32R)
    din("poseT", [9, BL], F32R)
    y = nc.dram_tensor("y", [1, BL], F32, kind="ExternalOutput")

    with tile.TileContext(nc) as tc:
        _build_body(nc, tc, dram, y, le, ld)
    nc.compile()
    return nc


def _build_body(nc, tc, dram, y_dram, le, ld):
    import contextlib
    ctx = contextlib.ExitStack()
    with ctx:
        ctx.enter_context(nc.allow_low_precision(
            reason="fp16 operands / approx reciprocal are intentional"))
        persist = ctx.enter_context(tc.tile_pool(name="persist", bufs=1))
        wpool = ctx.enter_context(tc.tile_pool(name="wpool", bufs=3))
        w2pool = ctx.enter_context(tc.tile_pool(name="w2pool", bufs=1))
        owpool = ctx.enter_context(tc.tile_pool(name="owpool", bufs=1))
        bpool = ctx.enter_context(tc.tile_pool(name="bpool", bufs=2))
        a4 = ctx.enter_context(tc.tile_pool(name="a4", bufs=3))
        a8 = ctx.enter_context(tc.tile_pool(name="a8", bufs=2))
        vp = ctx.enter_context(tc.tile_pool(name="vp", bufs=1))
        vpd = ctx.enter_context(tc.tile_pool(name="vpd", bufs=1))
        rtp = ctx.enter_context(tc.tile_pool(name="rtp", bufs=3))
        vbp = ctx.enter_context(tc.tile_pool(name="vbp", bufs=1))
        scr = ctx.enter_context(tc.tile_pool(name="scr", bufs=2))
        scrrc = ctx.enter_context(tc.tile_pool(name="scrrc", bufs=1))
        smalls = ctx.enter_context(tc.tile_pool(name="smalls", bufs=2))
        b1 = ctx.enter_context(tc.tile_pool(name="b1", bufs=4, space="PSUM"))
        b2 = ctx.enter_context(tc.tile_pool(name="b2", bufs=2, space="PSUM"))
        lnp = ctx.enter_context(tc.tile_pool(name="lnp", bufs=1, space="PSUM"))
        sqp = ctx.enter_context(tc.tile_pool(name="sqp", bufs=2))
        qk2p, atp = a8, a4  # share slots/tags

        # ---------------- persistent tiles ----------------
        x = persist.tile([128, KC, T], F16)           # residual stream (X.T)
        vloc_d = persist.tile([128, KC, 8, 65], F32R)  # decoder V (no pool)
        gridc = persist.tile([128, 2, S], F16)
        grids = persist.tile([128, 2, S], F16)
        ones128h = persist.tile([128, 1], F16)
        ones_row = persist.tile([1, 128], F32R)
        ones_rowh = persist.tile([1, 128], F16)
        eps_t = persist.tile([1, 1], F32)
        p = persist.tile([128, KC, BL], F16)          # decoder latent p.T
        nc.sync.dma_start(gridc[:], dram["gridc"][:])
        nc.sync.dma_start(grids[:], dram["grids"][:])
        ones8h = persist.tile([128, 8], F16)
        ones8r = persist.tile([128, 8], F32R)
        stage_f32 = rtp.tile([128, 128], F32, tag="rt")
        nc.vector.memset(stage_f32[:], 1.0)
        nc.vector.tensor_copy(ones128h[:], stage_f32[:, 0:1])
        nc.vector.tensor_copy(ones_row[:], stage_f32[0:1, :])
        nc.vector.tensor_copy(ones_rowh[:], stage_f32[0:1, :])
        nc.vector.tensor_copy(ones8h[:], stage_f32[:, 0:8])
        nc.vector.tensor_copy(ones8r[:], stage_f32[:, 0:8])
        nc.vector.memset(eps_t[:], LN_EPS)

        def c32(ap):
            return ap.bitcast(F32)

        def rstd_from_var(t2):
            """t2 (f32 [1,n]) <- 1/sqrt(t2+eps), via exp(-0.5*ln(t2+eps)).

            Keeps ACT on the natural_log_exp table set (no sqrt set swap)."""
            nc.scalar.activation(t2, t2, AF.Ln, bias=eps_t[:])
            nc.scalar.activation(t2, t2, AF.Exp, scale=-0.5)

        def ln(x_tile, sl, n_tok, h_out, out_sl):
            """h_out[:, :, out_sl] = LayerNorm_features(x_tile[:, :, sl])."""
            sq = a4.tile([128, KC, n_tok], F16, tag="a4")
            for k in range(KC):
                nc.vector.tensor_tensor(sq[:, k, :], x_tile[:, k, sl],
                                        x_tile[:, k, sl], ALU.mult)
            sum_ps = b2.tile([1, n_tok], F32, tag="b2")
            sq_ps = b2.tile([1, n_tok], F32, tag="b2")
            for k in range(KC):
                nc.tensor.matmul(sum_ps[:], ones128h[:], x_tile[:, k, sl],
                                 start=(k == 0), stop=(k == KC - 1))
            for k in range(KC):
                nc.tensor.matmul(sq_ps[:], ones128h[:], sq[:, k, :],
                                 start=(k == 0), stop=(k == KC - 1))
            ms = scr.tile([1, n_tok], F32, tag="scr")
            t2 = scr.tile([1, n_tok], F32, tag="scr")
            rc = scrrc.tile([1, 2, n_tok], F32R, tag="scr_rc")
            nc.scalar.activation(ms[:], sum_ps[:], AF.Copy, scale=1.0 / D)
            nc.vector.tensor_tensor(t2[:], ms[:], ms[:], ALU.mult)      # m^2
            nc.vector.scalar_tensor_tensor(
                t2[:], sq_ps[:], 1.0 / D, t2[:], ALU.mult, ALU.subtract)
            rstd_from_var(t2[:])
            nc.vector.tensor_copy(rc[:, 0, :], t2[:])
            nc.vector.scalar_tensor_tensor(
                rc[:, 1, :], ms[:], -1.0, rc[:, 0, :], ALU.mult, ALU.mult)  # c
            # fp32r misbehaves at tiny moving dims -- bitcast to plain f32
            # for n_tok < 256 (baseline did this via cv()).
            cv = (lambda ap: ap) if n_tok >= 256 else c32
            r_ps = b1.tile([128, n_tok], F32, tag="b1")
            c_ps = b1.tile([128, n_tok], F32, tag="b1")
            nc.tensor.matmul(r_ps[:], cv(ones_row[:]), cv(rc[:, 0, :]),
                             start=True, stop=True)
            nc.tensor.matmul(c_ps[:], cv(ones_row[:]), cv(rc[:, 1, :]),
                             start=True, stop=True)
            for k in range(KC):
                nc.vector.tensor_tensor(h_out[:, k, out_sl], x_tile[:, k, sl],
                                        r_ps[:], ALU.mult)
                nc.vector.tensor_tensor(h_out[:, k, out_sl], h_out[:, k, out_sl],
                                        c_ps[:], ALU.add)

        def ln_stats_pair(x_tile):
            """One LN chain for both samples: returns rc [1, 2, T] f32r."""
            sq = a8.tile([128, KC, T], F16, tag="a8")
            for k in range(KC):
                nc.vector.tensor_tensor(sq[:, k, :], x_tile[:, k, :],
                                        x_tile[:, k, :], ALU.mult)
            sum_ps = b2.tile([1, T], F32, tag="b2")
            sq_ps = b2.tile([1, T], F32, tag="b2")
            for nh in range(BL):
                nsl = slice(nh * S, (nh + 1) * S)
                for k in range(KC):
                    nc.tensor.matmul(sum_ps[:, nsl], ones128h[:],
                                     x_tile[:, k, nsl],
                                     start=(k == 0), stop=(k == KC - 1))
                for k in range(KC):
                    nc.tensor.matmul(sq_ps[:, nsl], ones128h[:], sq[:, k, nsl],
                                     start=(k == 0), stop=(k == KC - 1))
            rc = scrrc.tile([1, 2, T], F32R, tag="scr_rcT")
            ms = rc[:, 1, :]                      # mean parked in the c slot
            t2 = scrrc.tile([1, T], F32, tag="scrT2")
            nc.scalar.activation(ms, sum_ps[:], AF.Copy, scale=1.0 / D)
            nc.vector.tensor_tensor(t2[:], ms, ms, ALU.mult)
            nc.vector.scalar_tensor_tensor(
                t2[:], sq_ps[:], 1.0 / D, t2[:], ALU.mult, ALU.subtract)
            rstd_from_var(t2[:])
            nc.vector.scalar_tensor_tensor(
                rc[:, 1, :], ms, -1.0, t2[:], ALU.mult, ALU.mult)
            nc.vector.tensor_copy(rc[:, 0, :], t2[:])
            return rc

        def ln_apply(rc, x_tile, s, h_out):
            sl = slice(s * S, (s + 1) * S)
            r_ps = b1.tile([128, S], F32, tag="b1")
            c_ps = b1.tile([128, S], F32, tag="b1")
            nc.tensor.matmul(r_ps[:], ones_row[:], rc[:, 0, sl],
                             start=True, stop=True)
            nc.tensor.matmul(c_ps[:], ones_row[:], rc[:, 1, sl],
                             start=True, stop=True)
            for k in range(KC):
                nc.vector.tensor_tensor(h_out[:, k, :], x_tile[:, k, sl],
                                        r_ps[:], ALU.mult)
                nc.vector.tensor_tensor(h_out[:, k, :], h_out[:, k, :],
                                        c_ps[:], ALU.add)

        # ---------------- morph projection -> x ----------------
        morpht = a8.tile([3, T], F32R, tag="a8")
        nc.sync.dma_start(morpht[:], dram["morphT"][:])
        mpw = rtp.tile([3, 512], F32R, tag="rt")
        mpb = rtp.tile([128, 4], F32, tag="rt")
        nc.sync.dma_start(mpw[:], dram["mpwT"][:])
        nc.sync.dma_start(mpb[:], dram["mpb"][:])
        for m in range(KC):
            for s in range(BL):
                ps = b1.tile([128, S], F32, tag="b1")
                nc.tensor.matmul(ps[:], mpw[:, m * 128:(m + 1) * 128],
                                 morpht[:, s * S:(s + 1) * S], start=True, stop=True)
                nc.scalar.activation(x[:, m, s * S:(s + 1) * S], ps[:], AF.Relu,
                                     bias=mpb[:, m:m + 1])

        # ---------------- pose projection -> p ----------------
        poset = rtp.tile([9, BL], F32R, tag="rt")
        ppw = rtp.tile([9, 512], F32R, tag="rt")
        ppb = rtp.tile([128, 4], F32, tag="rt")
        nc.sync.dma_start(poset[:], dram["poseT"][:])
        nc.sync.dma_start(ppw[:], dram["ppwT"][:])
        nc.sync.dma_start(ppb[:], dram["ppb"][:])
        pps = b1.tile([128, KC, BL], F32, tag="b1")
        for m in range(KC):
            nc.tensor.matmul(pps[:, m, :], c32(ppw[:, m * 128:(m + 1) * 128]),
                             c32(poset[:]), start=True, stop=True)
        for m in range(KC):
            nc.scalar.activation(p[:, m, :], pps[:, m, :], AF.Relu,
                                 bias=ppb[:, m:m + 1])

        # ------------- encoder layers (sample-pipelined LN) -------------
        # Per-sample LN stats are issued inside the previous block's
        # instruction stream (deferred via `pend`), so the small stats
        # chain overlaps another sample's PE work and the PE never idles
        # across a LayerNorm boundary (keeps the HAM clock warm).
        pend = []

        def flush_pend():
            while pend:
                pend.pop(0)()

        def ln_sums(s):
            sl = slice(s * S, (s + 1) * S)
            sq = sqp.tile([128, KC, S], F16, tag="sq")
            for k in range(KC):
                nc.vector.tensor_tensor(sq[:, k, :], x[:, k, sl],
                                        x[:, k, sl], ALU.mult)
            sum_ps = lnp.tile([1, S], F32, tag="lnsum")
            sq_ps = lnp.tile([1, S], F32, tag="lnsq")
            for k in range(KC):
                nc.tensor.matmul(sum_ps[:], ones128h[:], x[:, k, sl],
                                 start=(k == 0), stop=(k == KC - 1))
            for k in range(KC):
                nc.tensor.matmul(sq_ps[:], ones128h[:], sq[:, k, :],
                                 start=(k == 0), stop=(k == KC - 1))
            return sum_ps, sq_ps

        def ln_finish(sums, tag):
            sum_ps, sq_ps = sums
            rc = scrrc.tile([1, 2, S], F32R, tag=tag)
            ms = rc[:, 1, :]
            t2 = scr.tile([1, S], F32, tag="lnt2")
            nc.scalar.activation(ms, sum_ps[:], AF.Copy, scale=1.0 / D)
            nc.vector.tensor_tensor(t2[:], ms, ms, ALU.mult)
            nc.vector.scalar_tensor_tensor(
                t2[:], sq_ps[:], 1.0 / D, t2[:], ALU.mult, ALU.subtract)
            rstd_from_var(t2[:])
            nc.vector.scalar_tensor_tensor(
                rc[:, 1, :], ms, -1.0, t2[:], ALU.mult, ALU.mult)
            nc.vector.tensor_copy(rc[:, 0, :], t2[:])
            return rc

        def ln_stats_one(s, tag):
            return ln_finish(ln_sums(s), tag)

        def ln_apply_one(rc, s, h_out):
            sl = slice(s * S, (s + 1) * S)
            r_ps = b1.tile([128, S], F32, tag="b1")
            c_ps = b1.tile([128, S], F32, tag="b1")
            nc.tensor.matmul(r_ps[:], ones_row[:], rc[:, 0, :],
                             start=True, stop=True)
            nc.tensor.matmul(c_ps[:], ones_row[:], rc[:, 1, :],
                             start=True, stop=True)
            for k in range(KC):
                nc.vector.tensor_tensor(h_out[:, k, :], x[:, k, sl],
                                        r_ps[:], ALU.mult)
                nc.vector.tensor_tensor(h_out[:, k, :], h_out[:, k, :],
                                        c_ps[:], ALU.add)

        rc_a = [ln_stats_one(s, f"rcA{s}") for s in range(BL)]
        rc_f = [None, None]

        for li in range(le):
            w1 = wpool.tile([128, KC, 3 * D], F16, tag="bigw")
            nc.sync.dma_start(w1[:], dram["ew1T"][li])
            w1b = bpool.tile([128, 12], F32, tag="w1b")
            nc.sync.dma_start(w1b[:], dram["ew1b"][li])
            w2 = w2pool.tile([128, KC, 2 * D], F16, tag="w2")
            nc.sync.dma_start(w2[:], dram["ew2T"][li])
            w2b = bpool.tile([128, 8], F32, tag="w2b")
            nc.sync.dma_start(w2b[:], dram["ew2b"][li])
            vbrow = bpool.tile([1, D], F16, tag="vbrow")
            nc.sync.dma_start(vbrow[:], dram["evb"][li])
            ow = owpool.tile([128, KC, D], F16, tag="ow")
            nc.sync.dma_start(ow[:], dram["eowT"][li])
            owb = bpool.tile([128, 4], F32, tag="owb")
            nc.sync.dma_start(owb[:], dram["eowb"][li])
            l1 = wpool.tile([128, KC, FF], F16, tag="bigw")
            nc.sync.dma_start(l1[:], dram["el1T"][li])
            l1b = bpool.tile([128, 16], F32, tag="l1b")
            nc.sync.dma_start(l1b[:], dram["el1b"][li])
            l2b = bpool.tile([128, 4], F32, tag="l2b")
            nc.sync.dma_start(l2b[:], dram["el2b"][li])

            # v-bias broadcast [128, 512] (token-major V bias), once per layer
            vb_ps = b1.tile([128, D], F32, tag="b1")
            nc.tensor.matmul(vb_ps[:], ones_rowh[:], vbrow[:], start=True, stop=True)
            vb_bc = vbp.tile([128, D], F32, tag="vb_bc")
            nc.scalar.activation(vb_bc[:], vb_ps[:], AF.Copy)

            for s in range(BL):
                sl = slice(s * S, (s + 1) * S)
                h = a4.tile([128, KC, S], F16, tag="a4")
                ln_apply_one(rc_a[s], s, h)
                # ---- stage 1: q,k (permuted) + v; rope interleaved so the
                # DVE starts rotating q/k chunks while stage-1 matmuls for
                # later chunks are still streaming (kills a recurring ~3.8us
                # PE stall at stage-2 waiting on qkr).
                qkv1 = a8.tile([128, 8, S], F16, tag="a8")
                qkr = a8.tile([128, 8, S], F16, tag="a8")
                v1 = a4.tile([128, KC, S], F16, tag="a4")

                def rope_half(half):
                    for c in range(2):
                        e = qkv1[:, half + c, :]
                        o = qkv1[:, half + 2 + c, :]
                        r1 = qkr[:, half + c, :]
                        r2 = qkr[:, half + 2 + c, :]
                        t1 = rtp.tile([128, S], F16, tag="rt")
                        nc.vector.tensor_tensor(r1, e, gridc[:, c, :], ALU.mult)
                        nc.vector.tensor_tensor(t1[:], o, grids[:, c, :], ALU.mult)
                        nc.vector.tensor_tensor(r1, r1, t1[:], ALU.subtract)
                        t2 = rtp.tile([128, S], F16, tag="rt")
                        nc.vector.tensor_tensor(r2, e, grids[:, c, :], ALU.mult)
                        nc.vector.tensor_tensor(t2[:], o, gridc[:, c, :], ALU.mult)
                        nc.vector.tensor_tensor(r2, r2, t2[:], ALU.add)

                for m in range(12):
                    ps = b1.tile([128, S], F32, tag="b1")
                    for k in range(KC):
                        nc.tensor.matmul(ps[:], w1[:, k, m * 128:(m + 1) * 128],
                                         h[:, k, :], start=(k == 0), stop=(k == KC - 1))
                    dest = qkv1[:, m, :] if m < 8 else v1[:, m - 8, :]
                    nc.vector.tensor_scalar_add(dest, ps[:], w1b[:, m:m + 1])
                    if m == 3:
                        rope_half(0)
                    elif m == 7:
                        rope_half(4)
                flush_pend()
                # ---- V before stage-2 (PE runway for the rope tail); V
                # evacs ride the idle GpSimd engine to keep DVE on rope ----
                vloc = vp.tile([128, KC, 8, 72], F16, tag="vloc")
                for t in range(KC):
                    nc.vector.tensor_copy(vloc[:, t, :, 64], ones8h[:])
                for t in range(KC):
                    ps = b1.tile([128, S], F32, tag="b1")
                    for k in range(KC):
                        nc.tensor.matmul(
                            ps[:], v1[:, k, t * 128:(t + 1) * 128],
                            w1[:, k, 2 * D:3 * D],
                            start=(k == 0), stop=(k == KC - 1))
                    nc.vector.tensor_tensor(
                        vloc[:, t, :, 0:64],
                        ps[:].rearrange("p (h d) -> p h d", h=H),
                        vb_bc[:].rearrange("p (h d) -> p h d", h=H), ALU.add)
                # ---- stage 2: Q,K ----
                qk2 = qk2p.tile([128, 8, S], F16, tag="a8")
                for m in range(8):
                    ps = b1.tile([128, S], F32, tag="b1")
                    base = 0 if m < 4 else 4
                    for k in range(KC):
                        nc.tensor.matmul(ps[:], w2[:, k, m * 128:(m + 1) * 128],
                                         qkr[:, base + k, :],
                                         start=(k == 0), stop=(k == KC - 1))
                    nc.scalar.activation(qk2[:, m, :], ps[:], AF.Identity,
                                         bias=w2b[:, m:m + 1])
                # ---- attention heads (paired: exp(h+1) hides under A@V(h)) ----
                o_t = a4.tile([128, KC, S], F16, tag="a4")
                for h0 in range(0, H, 2):
                    ats = {}
                    for hh in (h0, h0 + 1):
                        rows = slice(64 * (hh % 2), 64 * (hh % 2) + 64)
                        at = atp.tile([128, KC, S], F16, tag="a4")
                        for c in range(KC):
                            scp = b1.tile([128, S], F32, tag="b1")
                            nc.tensor.matmul(
                                scp[:],
                                qk2[rows, 4 + hh // 2, c * 128:(c + 1) * 128],
                                qk2[rows, hh // 2, :], start=True, stop=True)
                            nc.scalar.activation(at[:, c, :], scp[:], AF.Exp,
                                                 scale=float(1.0 / np.sqrt(DH)))
                        ats[hh] = at
                    for hh in (h0, h0 + 1):
                        rows = slice(64 * (hh % 2), 64 * (hh % 2) + 64)
                        at = ats[hh]
                        ov = b2.tile([65, S], F32, tag="b2")
                        for c in range(KC):
                            nc.tensor.matmul(ov[:], vloc[:, c, hh, 0:65],
                                             at[:, c, :],
                                             start=(c == 0), stop=(c == KC - 1))
                        # custom DVE recip mishandles nonzero psum base
                        # partition: stage the denominator row to SBUF p0
                        # via ACT Copy (regular op) first.
                        t_den = scr.tile([1, S], F32, tag="scr")
                        nc.scalar.activation(t_den[:], ov[64:65, :], AF.Copy)
                        rec = scr.tile([1, S], F32, tag="scr")
                        nc.vector.reciprocal_approx_fast(rec[:], t_den[:])
                        rec_h = scr.tile([1, S], F16, tag="scr")
                        nc.vector.tensor_copy(rec_h[:], rec[:])
                        rb = b2.tile([64, S], F32, tag="b2")
                        nc.tensor.matmul(rb[:], ones_rowh[:, 0:64], rec_h[:],
                                         start=True, stop=True)
                        rb_sb = scr.tile([64, S], F16, tag="scr")
                        nc.scalar.activation(rb_sb[:], rb[:], AF.Copy)
                        nc.vector.tensor_tensor(o_t[rows, hh // 2, :],
                                                ov[0:64, :], rb_sb[:], ALU.mult)
                # ---- out-proj + residual ----
                for m in range(KC):
                    ps = b1.tile([128, S], F32, tag="b1")
                    for k in range(KC):
                        nc.tensor.matmul(ps[:], ow[:, k, m * 128:(m + 1) * 128],
                                         o_t[:, k, :], start=(k == 0),
                                         stop=(k == KC - 1))
                    nc.vector.scalar_tensor_tensor(
                        x[:, m, sl], ps[:], owb[:, m:m + 1], x[:, m, sl],
                        ALU.add, ALU.add)

                def _mkf(s=s):
                    rc_f[s] = ln_stats_one(s, f"rcF{s}")
                pend.append(_mkf)
            # ---- phase B: l2 streams in (chunked) once w1 is released ----
            l2 = wpool.tile([128, FC, D], F16, tag="bigw")
            for kf in range(FC):
                nc.sync.dma_start(l2[:, kf, :], dram["el2T"][li][:, kf, :])
            for s in range(BL):
                sl = slice(s * S, (s + 1) * S)
                h2 = a4.tile([128, KC, S], F16, tag="a4")
                ln_apply_one(rc_f[s], s, h2)
                # ---- FFN ----
                f2 = [b1.tile([128, S], F32, tag="b1", name=f"f2_{_m}")
                      for _m in range(KC)]
                for kf in range(FC):
                    if kf == 4:
                        flush_pend()
                    f1 = b2.tile([128, S], F32, tag="b2")
                    for k in range(KC):
                        nc.tensor.matmul(f1[:], l1[:, k, kf * 128:(kf + 1) * 128],
                                         h2[:, k, :], start=(k == 0),
                                         stop=(k == KC - 1))
                    rt = rtp.tile([128, S], F16, tag="rt")
                    nc.vector.tensor_scalar(rt[:], f1[:], l1b[:, kf:kf + 1], 0.0,
                                            ALU.add, ALU.max)
                    for m in range(KC):
                        nc.tensor.matmul(f2[m][:], l2[:, kf, m * 128:(m + 1) * 128],
                                         rt[:], start=(kf == 0), stop=(kf == FC - 1))
                for m in range(KC):
                    nc.vector.scalar_tensor_tensor(
                        x[:, m, sl], f2[m][:], l2b[:, m:m + 1], x[:, m, sl],
                        ALU.add, ALU.add)

                def _mka(s=s):
                    rc_a[s] = ln_stats_one(s, f"rcA{s}")
                pend.append(_mka)

        # ---------------- final encoder LN (in-place; affine folded) --------
        me = x
        flush_pend()
        for s in range(BL):
            ln_apply_one(rc_a[s], s, x[:, :, s * S:(s + 1) * S])

        # ---------------- decoder layers ----------------
        for li in range(ld):
            dw = wpool.tile([128, KC, 3 * D], F16, tag="bigw")
            nc.sync.dma_start(dw[:], dram["dinT"][li])
            dwb = bpool.tile([128, 12], F32, tag="w1b")
            nc.sync.dma_start(dwb[:], dram["dinb"][li])
            dvbrow = bpool.tile([1, D], F16, tag="vbrow")
            nc.sync.dma_start(dvbrow[:], dram["dvb"][li])
            do = owpool.tile([128, KC, D], F32R, tag="ow")
            nc.sync.dma_start(do[:], dram["dowT"][li])
            dob = bpool.tile([128, 4], F32, tag="owb")
            nc.sync.dma_start(dob[:], dram["dowb"][li])
            m1 = wpool.tile([128, KC, M], F16, tag="bigw")
            nc.sync.dma_start(m1[:], dram["dm1T"][li])
            m1b = bpool.tile([128, 16], F32, tag="l1b")
            nc.sync.dma_start(m1b[:], dram["dm1b"][li])
            m2b = bpool.tile([128, 4], F32, tag="l2b")
            nc.sync.dma_start(m2b[:], dram["dm2b"][li])

            vb_ps = b1.tile([128, D], F32, tag="b1")
            nc.tensor.matmul(vb_ps[:], ones_rowh[:], dvbrow[:], start=True, stop=True)
            vb_bc = vbp.tile([128, D], F32, tag="vb_bc")
            nc.scalar.activation(vb_bc[:], vb_ps[:], AF.Copy)

            # LN(p) -> q_ln ; Q projection (all samples at once, N=BL)
            q_ln = smalls.tile([128, KC, 8], F16, tag="q_ln")
            ln(p, slice(None), BL, q_ln, slice(0, BL))
            qps = b1.tile([128, KC, BL], F32, tag="b1")
            for m in range(KC):
                for k in range(KC):
                    nc.tensor.matmul(qps[:, m, :],
                                     dw[:, k, m * 128:(m + 1) * 128],
                                     q_ln[:, k, 0:BL], start=(k == 0),
                                     stop=(k == KC - 1))
            q_sb = smalls.tile([128, KC, BL], F32R, tag="q_sb")
            for m in range(KC):
                nc.scalar.activation(q_sb[:, m, :], qps[:, m, :], AF.Identity,
                                     bias=dwb[:, m:m + 1])
            o_d = smalls.tile([128, KC, BL], F32R, tag="o_d")
            for s in range(BL):
                sl = slice(s * S, (s + 1) * S)
                # K (feature-major) and V' (token-major) over morph_enc
                k_sb = a4.tile([128, KC, S], F32R, tag="a4")
                for m in range(KC):
                    ps = b1.tile([128, S], F32, tag="b1")
                    for k in range(KC):
                        nc.tensor.matmul(
                            ps[:], dw[:, k, D + m * 128:D + (m + 1) * 128],
                            me[:, k, sl], start=(k == 0), stop=(k == KC - 1))
                    nc.scalar.activation(k_sb[:, m, :], ps[:], AF.Identity,
                                         bias=dwb[:, 4 + m:5 + m])
                vloc = vloc_d
                for t in range(KC):
                    nc.vector.tensor_copy(vloc[:, t, :, 64], ones8r[:])
                for t in range(KC):
                    ps = b1.tile([128, S], F32, tag="b1")
                    for k in range(KC):
                        nc.tensor.matmul(
                            ps[:], me[:, k, s * S + t * 128:s * S + (t + 1) * 128],
                            dw[:, k, 2 * D:3 * D],
                            start=(k == 0), stop=(k == KC - 1))
                    nc.vector.tensor_tensor(
                        vloc[:, t, :, 0:64],
                        ps[:].rearrange("p (h d) -> p h d", h=H),
                        vb_bc[:].rearrange("p (h d) -> p h d", h=H), ALU.add)
                scp = b1.tile([128, KC, H], F32, tag="b1")
                for hh in range(H):
                    rows = slice(64 * (hh % 2), 64 * (hh % 2) + 64)
                    for c in range(KC):
                        nc.tensor.matmul(
                            scp[:, c, hh:hh + 1],
                            c32(k_sb[rows, hh // 2, c * 128:(c + 1) * 128]),
                            c32(q_sb[rows, hh // 2, s:s + 1]),
                            start=True, stop=True)
                at = smalls.tile([128, KC, H], F32R, tag="at_d")
                nc.scalar.activation(at[:], scp[:], AF.Exp,
                                     scale=float(1.0 / np.sqrt(DH)))
                ov = b2.tile([65, H], F32, tag="b2")
                for hh in range(H):
                    for c in range(KC):
                        nc.tensor.matmul(ov[:, hh:hh + 1], c32(vloc[:, c, hh, :]),
                                         c32(at[:, c, hh:hh + 1]),
                                         start=(c == 0), stop=(c == KC - 1))
                rec = scr.tile([1, H], F32R, tag="scr")
                nc.vector.reciprocal(rec[:], ov[64:65, :])
                rb = b2.tile([64, H], F32, tag="b2")
                nc.tensor.matmul(rb[:], c32(ones_row[:, 0:64]), c32(rec[:]),
                                 start=True, stop=True)
                rb_sb = scr.tile([64, H], F32, tag="scr")
                nc.scalar.activation(rb_sb[:], rb[:], AF.Copy)
                for hh in range(H):
                    rows = slice(64 * (hh % 2), 64 * (hh % 2) + 64)
                    nc.vector.tensor_tensor(o_d[rows, hh // 2, s:s + 1],
                                            ov[0:64, hh:hh + 1],
                                            rb_sb[:, hh:hh + 1], ALU.mult)
            # out-proj + residual into p
            ops = b1.tile([128, KC, BL], F32, tag="b1")
            for m in range(KC):
                for k in range(KC):
                    nc.tensor.matmul(ops[:, m, :],
                                     c32(do[:, k, m * 128:(m + 1) * 128]),
                                     c32(o_d[:, k, :]), start=(k == 0),
                                     stop=(k == KC - 1))
            for m in range(KC):
                nc.vector.scalar_tensor_tensor(
                    p[:, m, :], ops[:, m, :], dob[:, m:m + 1], p[:, m, :],
                    ALU.add, ALU.add)
            # FFN on p (m2 streams in chunked once dw releases its slot)
            m2 = wpool.tile([128, MC, D], F16, tag="bigw")
            for kf in range(MC):
                nc.sync.dma_start(m2[:, kf, :], dram["dm2T"][li][:, kf, :])
            h2d = smalls.tile([128, KC, 8], F16, tag="q_ln")
            ln(p, slice(None), BL, h2d, slice(0, BL))
            mh = smalls.tile([128, MC, 8], F16, tag="mh")
            for mm_ in range(MC):
                ps = b1.tile([128, BL], F32, tag="b1")
                for k in range(KC):
                    nc.tensor.matmul(ps[:], m1[:, k, mm_ * 128:(mm_ + 1) * 128],
                                     h2d[:, k, 0:BL], start=(k == 0),
                                     stop=(k == KC - 1))
                nc.vector.tensor_scalar(mh[:, mm_, 0:BL], ps[:],
                                        m1b[:, mm_:mm_ + 1],
                                        0.0, ALU.add, ALU.max)
            m2ps = b1.tile([128, KC, BL], F32, tag="b1")
            for m in range(KC):
                for kf in range(MC):
                    nc.tensor.matmul(m2ps[:, m, :],
                                     m2[:, kf, m * 128:(m + 1) * 128],
                                     mh[:, kf, 0:BL], start=(kf == 0),
                                     stop=(kf == MC - 1))
            for m in range(KC):
                nc.vector.scalar_tensor_tensor(
                    p[:, m, :], m2ps[:, m, :], m2b[:, m:m + 1], p[:, m, :],
                    ALU.add, ALU.add)

        # ---------------- head (sigmoid via exp + recip; no table swap) -----
        hw = smalls.tile([128, KC, 8], F16, tag="hw")
        hbn = smalls.tile([1, 1], F32, tag="hb")
        nc.sync.dma_start(hw[:], dram["hwT"][:])
        nc.sync.dma_start(hbn[:], dram["hbn"][:])
        hg = smalls.tile([128, KC, 8], F16, tag="q_ln")
        ln(p, slice(None), BL, hg, slice(0, BL))
        hps = b2.tile([1, BL], F32, tag="b2")
        for k in range(KC):
            nc.tensor.matmul(hps[:], hw[:, k, 0:1], hg[:, k, 0:BL],
                             start=(k == 0), stop=(k == KC - 1))
        e_t = smalls.tile([1, BL], F32, tag="y_e")
        nc.scalar.activation(e_t[:], hps[:], AF.Exp, scale=-1.0, bias=hbn[:])
        nc.vector.tensor_scalar_add(e_t[:], e_t[:], 1.0)
        y_sb = smalls.tile([1, BL], F32, tag="y_sb")
        nc.vector.reciprocal_approx_fast(y_sb[:], e_t[:])
        nc.sync.dma_start(y_dram[:], y_sb[:])


# ----------------------------------------------------------------------------
# entry point
# ----------------------------------------------------------------------------

_NC_CACHE = {}


def kernel(**inputs):
    return _run(inputs, LE, LD)


def _run(inputs, le, ld, trace=False):
    w = prep_weights(inputs, le, ld)
    morph = np.asarray(inputs["morph"], np.float32)
    pose = np.asarray(inputs["pose"], np.float32)
    in_maps = []
    for c in range(NCORES):
        im = dict(w)
        mo = morph[c * BL:(c + 1) * BL]                 # [BL, S, 3]
        im["morphT"] = np.ascontiguousarray(
            mo.transpose(2, 0, 1).reshape(3, T))
        im["poseT"] = np.ascontiguousarray(pose[c * BL:(c + 1) * BL].T)
        in_maps.append(im)

    if ("nc", le, ld) not in _NC_CACHE:
        _NC_CACHE[("nc", le, ld)] = build(le, ld)
    nc = _NC_CACHE[("nc", le, ld)]
    res = run_bass_kernel_spmd(nc, in_maps, core_ids=list(range(NCORES)),
                               trace=trace)
    out = np.zeros((B, 1), np.float32)
    for c in range(NCORES):
        out[c * BL:(c + 1) * BL, 0] = res.results[c]["y"][0]
    if trace:
        return out, res
    return out
